# revision 24
# baseline (speedup 1.0000x reference)
"""TRN2 Bass kernel: 16-head MHA (B=2, S=2048, H=1024) sharded over 8 NeuronCores.

Sharding: data-parallel over batch (2) x tensor-parallel over head groups
(4 groups of 4 heads). Each core computes its 4 heads' attention for its batch
and a partial output projection; the host sums the 4 partials per batch,
transposes, and adds the output bias.

v3: fully fused single-phase pipeline.
  - QK^T head pairs issued as concurrent row-tiles ((0,0)/(64,0)); AV pairs as
    col-tiles ((0,0)/(0,64)); softmax denominators from a 4-way col-tiled pass
    of M=1 ones-matmuls accumulating in a dedicated PSUM bank.
  - exp() is one [128, 1024] ACTIVATE per (qb, kc, pair); 1/rowsum via
    reciprocal_approx_fast; partition-broadcast via a bf16 DRAM round trip;
    normalization multiplies PSUM x directly (no intermediate xu).
  - Projections are fused into the attention stream: inputs arrive as
    column-chunk DMAs on three queues; Q/K projection chunks and per-kc V
    projections run as PE filler inside q-block 0/1/2, sharing one PSUM bank
    ring with the output projection.
  - Output projection chunks trail one q-block behind; y is written bf16
    (host sums the 4 partials per batch in fp32).
"""

import sys

sys.path.insert(0, "/opt/trn_rl_repo")

from collections import deque
from contextlib import ExitStack

import numpy as np
import ml_dtypes

import concourse.tile as tile
from concourse import bacc, mybir

BF16 = mybir.dt.bfloat16
F32 = mybir.dt.float32
P = 128

LAG = 4            # kc-instances by which AV/rowsum matmuls trail QK/exp/mask
USE_GPSIMD_MASK = True   # offload 1/4 of mask multiplies to GPSIMD
OPROJ_EVERY = 2    # pop one oproj chunk every N kc-instances

_PROGRAM_CACHE = {}


def build_mha_program(S=2048, HID=1024, NH=4, DK=64, QB=512, aug=False):
    """Build + compile the per-core SPMD Bass program."""
    D = NH * DK
    assert NH == 4 and DK == 64
    SH = S // P                 # 16 key blocks
    HT = HID // P               # 8 hidden blocks
    HTa = HT + (1 if aug else 0)
    QBn = S // QB               # 4 q-blocks
    NPAIR = NH // 2             # 2 head pairs
    CH = S // QB                # 4 input column chunks (same size as QB)

    nc = bacc.Bacc("TRN2", target_bir_lowering=False, debug=False)

    qT_d = nc.dram_tensor("qT", [HTa * P, S], BF16, kind="ExternalInput").ap()
    kT_d = nc.dram_tensor("kT", [HTa * P, S], BF16, kind="ExternalInput").ap()
    vT_d = nc.dram_tensor("vT", [HTa * P, S], BF16, kind="ExternalInput").ap()
    maskT_d = nc.dram_tensor("maskT", [S, S], BF16, kind="ExternalInput").ap()
    wq_d = nc.dram_tensor("wq", [HTa * P, D], BF16, kind="ExternalInput").ap()
    wk_d = nc.dram_tensor("wk", [HTa * P, D], BF16, kind="ExternalInput").ap()
    wv_d = nc.dram_tensor("wv", [HTa * P, D], BF16, kind="ExternalInput").ap()
    wo_d = nc.dram_tensor("wo", [D, HID], BF16, kind="ExternalInput").ap()
    y_d = nc.dram_tensor("y", [HID, S], BF16, kind="ExternalOutput").ap()
    # DRAM bounce buffer for partition-broadcasting the softmax reciprocals
    rb_d = nc.dram_tensor("r_bounce", [NH * QBn, QB], BF16).ap()

    Exp = mybir.ActivationFunctionType.Exp

    with tile.TileContext(nc) as tc:
        with ExitStack() as ctx:
            persist = ctx.enter_context(tc.tile_pool(name="persist", bufs=1))
            wpool = ctx.enter_context(tc.tile_pool(name="wpool", bufs=1))
            inq = ctx.enter_context(tc.tile_pool(name="inq", bufs=2))
            mp = ctx.enter_context(tc.tile_pool(name="mask", bufs=1))
            pp = ctx.enter_context(tc.tile_pool(name="pexp", bufs=2))
            pmp = ctx.enter_context(
                tc.tile_pool(name="pmask", bufs=2 * (LAG + 2)))
            rfp = ctx.enter_context(tc.tile_pool(name="rfp", bufs=2))
            rbp = ctx.enter_context(tc.tile_pool(name="rbp", bufs=2))
            ysb = ctx.enter_context(tc.tile_pool(name="ysb", bufs=2))
            sps = ctx.enter_context(
                tc.tile_pool(name="sps", bufs=2, space="PSUM"))
            xps = ctx.enter_context(
                tc.tile_pool(name="xps", bufs=1, space="PSUM"))
            rsps = ctx.enter_context(
                tc.tile_pool(name="rsps", bufs=1, space="PSUM"))
            yps = ctx.enter_context(
                tc.tile_pool(name="yps", bufs=1, space="PSUM"))

            qh_t = [persist.tile([P, S], BF16, tag=f"qh{d}", name=f"qh{d}")
                    for d in range(NPAIR)]
            kh_t = [persist.tile([P, S], BF16, tag=f"kh{d}", name=f"kh{d}")
                    for d in range(NPAIR)]
            vh_t = [persist.tile([P, D], BF16, tag=f"vh{s}", name=f"vh{s}")
                    for s in range(SH)]
            xn_t = [persist.tile([P, S], BF16, tag=f"xn{p}", name=f"xn{p}")
                    for p in range(NPAIR)]
            wo_t = [persist.tile([P, HID], BF16, tag=f"wo{p}", name=f"wo{p}")
                    for p in range(NPAIR)]
            ones_t = persist.tile([P, 4], BF16, tag="ones", name="ones")
            nc.vector.memset(ones_t[:], 1.0)

            wq_t = wpool.tile([P, HTa * D], BF16, tag="wq", name="wq")
            wk_t = wpool.tile([P, HTa * D], BF16, tag="wk", name="wk")
            wv_t = wpool.tile([P, HTa * D], BF16, tag="wv", name="wv")

            mask_t = [mp.tile([P, S], BF16, tag=f"m{i}", name=f"m{i}")
                      for i in range(SH)]

            x_ps = [xps.tile([P, QB], F32, tag=f"x{p}", name=f"x{p}")
                    for p in range(NPAIR)]
            rs_ps = rsps.tile([P, QB], F32, tag="rs", name="rs")

            def wload(eng, dst_t, src_d):
                dst = dst_t[:].rearrange("p (i s) -> p i s", s=D)
                src = src_d[:, :].rearrange("(i p) s -> p i s", p=P)
                eng.dma_start(dst, src)

            def chunk_load(eng, tag, src_d, c):
                """Column chunk c (QB cols) of all HTa row-blocks."""
                t = inq.tile([P, HTa * QB], BF16, tag=tag, name=f"{tag}{c}")
                dst = t[:].rearrange("p (i s) -> p i s", s=QB)
                src = src_d[:, c * QB:(c + 1) * QB].rearrange(
                    "(i p) s -> p i s", p=P)
                eng.dma_start(dst, src)
                return t

            # -------- DMA schedule (3 queues: sync / scalar / gpsimd) -------
            # Head of pipeline: kT0 alone on scalar (gates kh-c0 proj),
            # wq+qT0 on sync (gates qh-c0 proj), wk/wv early on gpsimd.
            qT_c, kT_c, vT_c = {}, {}, {}
            kT_c[0] = chunk_load(nc.scalar, "kT", kT_d, 0)
            wload(nc.sync, wq_t, wq_d)
            qT_c[0] = chunk_load(nc.sync, "qT", qT_d, 0)
            wload(nc.gpsimd, wk_t, wk_d)
            wload(nc.gpsimd, wv_t, wv_d)
            kT_c[1] = chunk_load(nc.scalar, "kT", kT_d, 1)
            nc.sync.dma_start(mask_t[0][:], maskT_d[0:P, :])
            vT_c[0] = chunk_load(nc.gpsimd, "vT", vT_d, 0)
            nc.sync.dma_start(mask_t[1][:], maskT_d[P:2 * P, :])
            qT_c[1] = chunk_load(nc.scalar, "qT", qT_d, 1)
            vT_c[1] = chunk_load(nc.gpsimd, "vT", vT_d, 1)
            nc.gpsimd.dma_start(wo_t[0][:], wo_d[0:P, :])
            nc.gpsimd.dma_start(wo_t[1][:], wo_d[P:2 * P, :])
            for i in range(2, 8):
                nc.sync.dma_start(mask_t[i][:], maskT_d[i * P:(i + 1) * P, :])

            def mload(eng, i):
                eng.dma_start(mask_t[i][:], maskT_d[i * P:(i + 1) * P, :])

            def late_loads():
                """Issued mid-stream on sync/gpsimd (never scalar: its queue
                carries the exp stream). Later tranches are staggered so the
                latency-critical rb/y DMAs interleave between them."""
                kT_c[2] = chunk_load(nc.gpsimd, "kT", kT_d, 2)
                vT_c[2] = chunk_load(nc.sync, "vT", vT_d, 2)
                kT_c[3] = chunk_load(nc.gpsimd, "kT", kT_d, 3)
                vT_c[3] = chunk_load(nc.gpsimd, "vT", vT_d, 3)
                for i in range(8, SH):
                    mload(nc.sync if i % 2 == 0 else nc.gpsimd, i)

            def late_loads2():
                qT_c[2] = chunk_load(nc.sync, "qT", qT_d, 2)

            def late_loads4():
                qT_c[3] = chunk_load(nc.sync, "qT", qT_d, 3)

            # ---------------- fused projection helpers ----------------
            def proj_chunk(src_c, w_t, dst, qc, eng, dcs=(0, 1)):
                """dst[:, qc chunk] = (w.T @ src) for the given row halves."""
                for dc in dcs:
                    ps = yps.tile([P, QB], F32, tag="y", name=f"pj{qc}{dc}")
                    for i in range(HTa):
                        nc.tensor.matmul(
                            ps[:],
                            w_t[:, i * D + dc * P:i * D + (dc + 1) * P],
                            src_c[:, i * QB:(i + 1) * QB],
                            start=(i == 0), stop=(i == HTa - 1))
                    dst_ap = dst[dc][:, qc * QB:(qc + 1) * QB]
                    if eng is nc.scalar:
                        eng.copy(dst_ap, ps[:])
                    else:
                        eng.tensor_copy(dst_ap, ps[:])

            def vh_proj(sc):
                """vh[sc] = vT[:, sc block].T @ wv  (one [P, D] tile)."""
                c = sc // (QB // P)
                off = (sc % (QB // P)) * P
                ps = yps.tile([P, QB], F32, tag="y", name=f"vj{sc}")
                for i in range(HTa):
                    nc.tensor.matmul(
                        ps[:, 0:D],
                        vT_c[c][:, i * QB + off:i * QB + off + P],
                        wv_t[:, i * D:(i + 1) * D],
                        start=(i == 0), stop=(i == HTa - 1))
                nc.vector.tensor_copy(vh_t[sc][:], ps[:, 0:D])

            # ---------------- attention pipeline ----------------
            def emit_qk_exp_mask(qb, kc):
                qsl = slice(qb * QB, (qb + 1) * QB)
                pms = []
                for pr in range(NPAIR):
                    s_ps = sps.tile([P, 2 * QB], F32, tag="s", name="s")
                    for hb in range(2):
                        rsl = slice(64 * hb, 64 * hb + 64)
                        nc.tensor.matmul(
                            s_ps[:, hb * QB:(hb + 1) * QB],
                            kh_t[pr][rsl, kc * P:(kc + 1) * P],
                            qh_t[pr][rsl, qsl],
                            start=True, stop=True)
                    p_t = pp.tile([P, 2 * QB], BF16, tag="p", name="p")
                    nc.scalar.activation(p_t[:], s_ps[:], Exp, scale=0.125)
                    pm = pmp.tile([P, 2 * QB], BF16, tag="pm", name="pm")
                    eng = nc.vector
                    if USE_GPSIMD_MASK and (pr == 1) and (kc % 2 == 1):
                        eng = nc.gpsimd
                    for hb in range(2):
                        eng.tensor_mul(
                            pm[:, hb * QB:(hb + 1) * QB],
                            p_t[:, hb * QB:(hb + 1) * QB],
                            mask_t[kc][:, qsl])
                    pms.append(pm)
                return pms

            def emit_av_rs(qb, kc, pms):
                for pr in range(NPAIR):
                    for hb in range(2):
                        h = 2 * pr + hb
                        nc.tensor.matmul(
                            x_ps[pr][64 * hb:64 * hb + 64, :],
                            vh_t[kc][:, h * DK:(h + 1) * DK],
                            pms[pr][:, hb * QB:(hb + 1) * QB],
                            start=(kc == 0), stop=(kc == SH - 1),
                            skip_group_check=True)
                for pr in range(NPAIR):
                    for hb in range(2):
                        h = 2 * pr + hb
                        nc.tensor.matmul(
                            rs_ps[32 * h:32 * h + 1, :],
                            ones_t[:, 0:1],
                            pms[pr][:, hb * QB:(hb + 1) * QB],
                            start=(kc == 0), stop=(kc == SH - 1),
                            skip_group_check=True,
                            tile_position=(0, 32 * h))

            def normalize(qb):
                """r = 1/rowsum, broadcast via DRAM, xn = x_ps * r."""
                qsl = slice(qb * QB, (qb + 1) * QB)
                r32 = rfp.tile([P, QB], F32, tag="r32", name="r32")
                r16 = rfp.tile([P, QB], BF16, tag="r16", name="r16")
                nc.vector.reciprocal_approx_fast(out=r32[:], in_=rs_ps[:])
                nc.vector.tensor_copy(r16[:], r32[:])
                rows = r16[:].rearrange("(g p) q -> g p q", p=32)[:, 0:1, :]
                nc.sync.dma_start(
                    rb_d[qb * NH:(qb + 1) * NH, :].rearrange(
                        "(g o) q -> g o q", o=1),
                    rows)
                for pr in range(NPAIR):
                    rb = rbp.tile([P, QB], BF16, tag="rb", name="rb")
                    for hb in range(2):
                        row = qb * NH + 2 * pr + hb
                        nc.sync.dma_start(
                            rb[64 * hb:64 * hb + 64, :],
                            rb_d[row:row + 1, :].broadcast_to([64, QB]))
                    nc.vector.tensor_mul(
                        xn_t[pr][:, qsl], x_ps[pr][:], rb[:])

            def oproj_chunk(qb, hc, pool):
                qsl = slice(qb * QB, (qb + 1) * QB)
                if pool is yps:
                    y_ps = pool.tile([P, QB], F32, tag="y", name="y")
                else:
                    y_ps = pool.tile([P, 2 * QB], F32, tag="s",
                                     name="y")[:, 0:QB]
                for pr in range(NPAIR):
                    nc.tensor.matmul(
                        y_ps[:],
                        wo_t[pr][:, hc * P:(hc + 1) * P],
                        xn_t[pr][:, qsl],
                        start=(pr == 0), stop=(pr == NPAIR - 1))
                y_sb = ysb.tile([P, QB], BF16, tag="ysb", name="ysb")
                nc.vector.tensor_copy(y_sb[:], y_ps[:])
                nc.sync.dma_start(y_d[hc * P:(hc + 1) * P, qsl], y_sb[:])

            # ---------------- emission schedule ----------------
            proj_chunk(kT_c[0], wk_t, kh_t, 0, nc.scalar)
            proj_chunk(qT_c[0], wq_t, qh_t, 0, nc.scalar)

            # (qb, kc) -> fused PE filler emitted just before that QK;
            # proj chunks split per-dc to halve the PE spike.
            filler = {
                (0, 1): lambda: late_loads(),
                (0, 2): lambda: proj_chunk(kT_c[1], wk_t, kh_t, 1, nc.vector,
                                           (0,)),
                (0, 3): lambda: proj_chunk(kT_c[1], wk_t, kh_t, 1, nc.vector,
                                           (1,)),
                (0, 6): lambda: proj_chunk(kT_c[2], wk_t, kh_t, 2, nc.vector,
                                           (0,)),
                (0, 7): lambda: proj_chunk(kT_c[2], wk_t, kh_t, 2, nc.vector,
                                           (1,)),
                (0, 10): lambda: proj_chunk(kT_c[3], wk_t, kh_t, 3, nc.vector,
                                            (0,)),
                (0, 11): lambda: proj_chunk(kT_c[3], wk_t, kh_t, 3, nc.vector,
                                            (1,)),
                (0, 13): lambda: proj_chunk(qT_c[1], wq_t, qh_t, 1, nc.vector,
                                            (0,)),
                (0, 14): lambda: proj_chunk(qT_c[1], wq_t, qh_t, 1, nc.vector,
                                            (1,)),
                (1, 1): lambda: late_loads2(),
                (1, 6): lambda: proj_chunk(qT_c[2], wq_t, qh_t, 2, nc.vector,
                                           (0,)),
                (1, 7): lambda: proj_chunk(qT_c[2], wq_t, qh_t, 2, nc.vector,
                                           (1,)),
                (1, 9): lambda: late_loads4(),
                (2, 2): lambda: proj_chunk(qT_c[3], wq_t, qh_t, 3, nc.vector,
                                           (0,)),
                (2, 3): lambda: proj_chunk(qT_c[3], wq_t, qh_t, 3, nc.vector,
                                           (1,)),
            }

            pending = deque()
            oproj_q = deque()

            def pop_pending():
                qb0, kc0, pms0 = pending.popleft()
                emit_av_rs(qb0, kc0, pms0)
                if kc0 == SH - 1:
                    normalize(qb0)
                    for hc in range(HT):
                        oproj_q.append((qb0, hc))

            t = 0
            for qb in range(QBn):
                for kc in range(SH):
                    if (qb, kc) in filler:
                        filler[(qb, kc)]()
                    pms = emit_qk_exp_mask(qb, kc)
                    if qb == 0:
                        vh_proj(kc)
                    pending.append((qb, kc, pms))
                    if len(pending) > LAG:
                        pop_pending()
                    if t % OPROJ_EVERY == 1 and oproj_q:
                        oproj_chunk(*oproj_q.popleft(), yps)
                    t += 1
            while pending:
                pop_pending()
            while oproj_q:
                oproj_chunk(*oproj_q.popleft(), sps)

    nc.compile()
    return nc


def make_in_maps(q, k, v, mask, Wq, bq, Wk, bk, Wv, bv, Wo,
                 n_cores=8, NH=4, DK=64, aug=False):
    bf = ml_dtypes.bfloat16
    B, S, HID = q.shape
    D = NH * DK
    n_hg = n_cores // B

    def with_aug(xT, bias_row):
        pad = np.zeros((P, xT.shape[1]), xT.dtype)
        pad[0, :] = bias_row
        return np.concatenate([xT, pad], axis=0)

    per_batch = {}
    for b in range(B):
        qT = np.ascontiguousarray(q[b].T).astype(bf)
        kT = np.ascontiguousarray(k[b].T).astype(bf)
        vT = np.ascontiguousarray(v[b].T).astype(bf)
        if aug:
            one = np.ones((S,), np.float32).astype(bf)
            qT, kT, vT = with_aug(qT, one), with_aug(kT, one), with_aug(vT, one)
        per_batch[b] = (qT, kT, vT,
                        np.ascontiguousarray(mask[b, 0].T != 0).astype(bf))

    in_maps = []
    for core in range(n_cores):
        b, hg = divmod(core, n_hg)
        hsl = slice(hg * D, (hg + 1) * D)
        wq = Wq[:, hsl].astype(bf)
        wk = Wk[:, hsl].astype(bf)
        wv = Wv[:, hsl].astype(bf)
        if aug:
            wq = with_aug(wq, bq[hsl].astype(bf))
            wk = with_aug(wk, bk[hsl].astype(bf))
            wv = with_aug(wv, bv[hsl].astype(bf))
        qT, kT, vT, mT = per_batch[b]
        in_maps.append(dict(
            qT=qT, kT=kT, vT=vT, maskT=mT,
            wq=np.ascontiguousarray(wq), wk=np.ascontiguousarray(wk),
            wv=np.ascontiguousarray(wv),
            wo=np.ascontiguousarray(Wo[hsl, :]).astype(bf),
        ))
    return in_maps


def combine_outputs(results, B, S, HID, bo, n_cores=8):
    n_hg = n_cores // B
    out = np.zeros((B, S, HID), np.float32)
    for core in range(n_cores):
        b = core // n_hg
        out[b] += results[core]["y"].astype(np.float32).T
    return out + bo.astype(np.float32)


def run_mha(q, k, v, mask, Wq, bq, Wk, bk, Wv, bv, Wo, bo, trace=False):
    from concourse.bass_utils import run_bass_kernel_spmd

    B, S, HID = q.shape
    n_cores = 8
    aug = bool(np.any(bq) or np.any(bk) or np.any(bv))
    key = (S, HID, aug)
    if key not in _PROGRAM_CACHE:
        _PROGRAM_CACHE[key] = build_mha_program(S=S, HID=HID, aug=aug)
    nc = _PROGRAM_CACHE[key]
    in_maps = make_in_maps(q, k, v, mask, Wq, bq, Wk, bk, Wv, bv, Wo,
                           n_cores=n_cores, aug=aug)
    res = run_bass_kernel_spmd(nc, in_maps, list(range(n_cores)), trace=trace)
    out = combine_outputs(res.results, B, S, HID, bo, n_cores=n_cores)
    return out, res


def kernel(q, k, v, mask, Wq, bq, Wk, bk, Wv, bv, Wo, bo):
    q = np.asarray(q, np.float32)
    k = np.asarray(k, np.float32)
    v = np.asarray(v, np.float32)
    mask = np.asarray(mask)
    out, _ = run_mha(q, k, v, mask,
                     np.asarray(Wq, np.float32), np.asarray(bq, np.float32),
                     np.asarray(Wk, np.float32), np.asarray(bk, np.float32),
                     np.asarray(Wv, np.float32), np.asarray(bv, np.float32),
                     np.asarray(Wo, np.float32), np.asarray(bo, np.float32))
    return out


# revision 27
# speedup vs baseline: 1.1214x; 1.1214x over previous
"""TRN2 Bass kernel: 16-head MHA (B=2, S=2048, H=1024) sharded over 8 NeuronCores.

Sharding: data-parallel over batch (2) x tensor-parallel over head groups
(4 groups of 4 heads). Each core computes its 4 heads' attention for its batch
and a partial output projection; the host sums the 4 partials per batch,
transposes, and adds the output bias.

v3: fully fused single-phase pipeline.
  - QK^T head pairs issued as concurrent row-tiles ((0,0)/(64,0)); AV pairs as
    col-tiles ((0,0)/(0,64)); softmax denominators from a 4-way col-tiled pass
    of M=1 ones-matmuls accumulating in a dedicated PSUM bank.
  - exp() is one [128, 1024] ACTIVATE per (qb, kc, pair); 1/rowsum via
    reciprocal_approx_fast; partition-broadcast via a bf16 DRAM round trip;
    normalization multiplies PSUM x directly (no intermediate xu).
  - Projections are fused into the attention stream: inputs arrive as
    column-chunk DMAs on three queues; Q/K projection chunks and per-kc V
    projections run as PE filler inside q-block 0/1/2, sharing one PSUM bank
    ring with the output projection.
  - Output projection chunks trail one q-block behind; y is written bf16
    (host sums the 4 partials per batch in fp32).
"""

import sys

sys.path.insert(0, "/opt/trn_rl_repo")

from collections import deque
from contextlib import ExitStack

import numpy as np
import ml_dtypes

import concourse.tile as tile
from concourse import bacc, mybir

BF16 = mybir.dt.bfloat16
F32 = mybir.dt.float32
P = 128

LAG = 4            # kc-instances by which AV/rowsum matmuls trail QK/exp/mask
USE_GPSIMD_MASK = True   # offload 1/4 of mask multiplies to GPSIMD
OPROJ_EVERY = 2    # pop one oproj chunk every N kc-instances

_PROGRAM_CACHE = {}


def build_mha_program(S=2048, HID=1024, NH=4, DK=64, QB=512, aug=False):
    """Build + compile the per-core SPMD Bass program."""
    D = NH * DK
    assert NH == 4 and DK == 64
    SH = S // P                 # 16 key blocks
    HT = HID // P               # 8 hidden blocks
    HTa = HT + (1 if aug else 0)
    QBn = S // QB               # 4 q-blocks
    NPAIR = NH // 2             # 2 head pairs
    CH = S // QB                # 4 input column chunks (same size as QB)

    nc = bacc.Bacc("TRN2", target_bir_lowering=False, debug=False)

    qT_d = nc.dram_tensor("qT", [HTa * P, S], BF16, kind="ExternalInput").ap()
    kT_d = nc.dram_tensor("kT", [HTa * P, S], BF16, kind="ExternalInput").ap()
    vT_d = nc.dram_tensor("vT", [HTa * P, S], BF16, kind="ExternalInput").ap()
    maskT_d = nc.dram_tensor("maskT", [S, S], BF16, kind="ExternalInput").ap()
    wq_d = nc.dram_tensor("wq", [HTa * P, D], BF16, kind="ExternalInput").ap()
    wk_d = nc.dram_tensor("wk", [HTa * P, D], BF16, kind="ExternalInput").ap()
    wv_d = nc.dram_tensor("wv", [HTa * P, D], BF16, kind="ExternalInput").ap()
    wo_d = nc.dram_tensor("wo", [D, HID], BF16, kind="ExternalInput").ap()
    y_d = nc.dram_tensor("y", [HID, S], BF16, kind="ExternalOutput").ap()
    # DRAM bounce buffer for partition-broadcasting the softmax reciprocals
    rb_d = nc.dram_tensor("r_bounce", [NH * QBn, QB], BF16).ap()

    Exp = mybir.ActivationFunctionType.Exp

    with tile.TileContext(nc) as tc:
        with ExitStack() as ctx:
            persist = ctx.enter_context(tc.tile_pool(name="persist", bufs=1))
            wpool = ctx.enter_context(tc.tile_pool(name="wpool", bufs=1))
            inq = ctx.enter_context(tc.tile_pool(name="inq", bufs=2))
            mp = ctx.enter_context(tc.tile_pool(name="mask", bufs=1))
            pp = ctx.enter_context(tc.tile_pool(name="pexp", bufs=2))
            pmp = ctx.enter_context(
                tc.tile_pool(name="pmask", bufs=2 * (LAG + 2)))
            rfp = ctx.enter_context(tc.tile_pool(name="rfp", bufs=2))
            rbp = ctx.enter_context(tc.tile_pool(name="rbp", bufs=2))
            ysb = ctx.enter_context(tc.tile_pool(name="ysb", bufs=2))
            sps = ctx.enter_context(
                tc.tile_pool(name="sps", bufs=2, space="PSUM"))
            xps = ctx.enter_context(
                tc.tile_pool(name="xps", bufs=1, space="PSUM"))
            rsps = ctx.enter_context(
                tc.tile_pool(name="rsps", bufs=1, space="PSUM"))
            yps = ctx.enter_context(
                tc.tile_pool(name="yps", bufs=1, space="PSUM"))

            qh_t = [persist.tile([P, S], BF16, tag=f"qh{d}", name=f"qh{d}")
                    for d in range(NPAIR)]
            kh_t = [persist.tile([P, S], BF16, tag=f"kh{d}", name=f"kh{d}")
                    for d in range(NPAIR)]
            vh_t = [persist.tile([P, D], BF16, tag=f"vh{s}", name=f"vh{s}")
                    for s in range(SH)]
            xn_t = [persist.tile([P, S], BF16, tag=f"xn{p}", name=f"xn{p}")
                    for p in range(NPAIR)]
            wo_t = [persist.tile([P, HID], BF16, tag=f"wo{p}", name=f"wo{p}")
                    for p in range(NPAIR)]
            ones_t = persist.tile([P, 4], BF16, tag="ones", name="ones")
            nc.vector.memset(ones_t[:], 1.0)

            wq_t = wpool.tile([P, HTa * D], BF16, tag="wq", name="wq")
            wk_t = wpool.tile([P, HTa * D], BF16, tag="wk", name="wk")
            wv_t = wpool.tile([P, HTa * D], BF16, tag="wv", name="wv")

            mask_t = [mp.tile([P, S], BF16, tag=f"m{i}", name=f"m{i}")
                      for i in range(SH)]

            x_ps = [xps.tile([P, QB], F32, tag=f"x{p}", name=f"x{p}")
                    for p in range(NPAIR)]
            rs_ps = rsps.tile([P, QB], F32, tag="rs", name="rs")

            def wload(eng, dst_t, src_d):
                dst = dst_t[:].rearrange("p (i s) -> p i s", s=D)
                src = src_d[:, :].rearrange("(i p) s -> p i s", p=P)
                eng.dma_start(dst, src)

            def chunk_load(eng, tag, src_d, c):
                """Column chunk c (QB cols) of all HTa row-blocks."""
                t = inq.tile([P, HTa * QB], BF16, tag=tag, name=f"{tag}{c}")
                dst = t[:].rearrange("p (i s) -> p i s", s=QB)
                src = src_d[:, c * QB:(c + 1) * QB].rearrange(
                    "(i p) s -> p i s", p=P)
                eng.dma_start(dst, src)
                return t

            # -------- DMA schedule (3 queues: sync / scalar / gpsimd) -------
            # Head of pipeline: kT0 alone on scalar (gates kh-c0 proj),
            # wq+qT0 on sync (gates qh-c0 proj), wk/wv early on gpsimd.
            qT_c, kT_c, vT_c = {}, {}, {}
            kT_c[0] = chunk_load(nc.scalar, "kT", kT_d, 0)
            wload(nc.sync, wq_t, wq_d)
            qT_c[0] = chunk_load(nc.sync, "qT", qT_d, 0)
            wload(nc.gpsimd, wk_t, wk_d)
            wload(nc.gpsimd, wv_t, wv_d)
            kT_c[1] = chunk_load(nc.scalar, "kT", kT_d, 1)
            nc.sync.dma_start(mask_t[0][:], maskT_d[0:P, :])
            vT_c[0] = chunk_load(nc.gpsimd, "vT", vT_d, 0)
            nc.sync.dma_start(mask_t[1][:], maskT_d[P:2 * P, :])
            qT_c[1] = chunk_load(nc.scalar, "qT", qT_d, 1)
            vT_c[1] = chunk_load(nc.gpsimd, "vT", vT_d, 1)
            nc.gpsimd.dma_start(wo_t[0][:], wo_d[0:P, :])
            nc.gpsimd.dma_start(wo_t[1][:], wo_d[P:2 * P, :])
            for i in range(2, 8):
                nc.sync.dma_start(mask_t[i][:], maskT_d[i * P:(i + 1) * P, :])

            def mload(eng, i):
                eng.dma_start(mask_t[i][:], maskT_d[i * P:(i + 1) * P, :])

            def late_loads():
                """Issued mid-stream on sync/gpsimd (never scalar: its queue
                carries the exp stream). Later tranches are staggered so the
                latency-critical rb/y DMAs interleave between them."""
                kT_c[2] = chunk_load(nc.gpsimd, "kT", kT_d, 2)
                vT_c[2] = chunk_load(nc.sync, "vT", vT_d, 2)
                kT_c[3] = chunk_load(nc.gpsimd, "kT", kT_d, 3)
                qT_c[2] = chunk_load(nc.sync, "qT", qT_d, 2)
                vT_c[3] = chunk_load(nc.gpsimd, "vT", vT_d, 3)
                qT_c[3] = chunk_load(nc.sync, "qT", qT_d, 3)
                for i in range(8, SH):
                    mload(nc.sync if i % 2 == 0 else nc.gpsimd, i)

            # ---------------- fused projection helpers ----------------
            def proj_chunk(src_c, w_t, dst, qc, eng, dcs=(0, 1)):
                """dst[:, qc chunk] = (w.T @ src) for the given row halves."""
                for dc in dcs:
                    ps = yps.tile([P, QB], F32, tag="y", name=f"pj{qc}{dc}")
                    for i in range(HTa):
                        nc.tensor.matmul(
                            ps[:],
                            w_t[:, i * D + dc * P:i * D + (dc + 1) * P],
                            src_c[:, i * QB:(i + 1) * QB],
                            start=(i == 0), stop=(i == HTa - 1))
                    dst_ap = dst[dc][:, qc * QB:(qc + 1) * QB]
                    if eng is nc.scalar:
                        eng.copy(dst_ap, ps[:])
                    else:
                        eng.tensor_copy(dst_ap, ps[:])

            def vh_proj(sc):
                """vh[sc] = vT[:, sc block].T @ wv  (one [P, D] tile)."""
                c = sc // (QB // P)
                off = (sc % (QB // P)) * P
                ps = yps.tile([P, QB], F32, tag="y", name=f"vj{sc}")
                for i in range(HTa):
                    nc.tensor.matmul(
                        ps[:, 0:D],
                        vT_c[c][:, i * QB + off:i * QB + off + P],
                        wv_t[:, i * D:(i + 1) * D],
                        start=(i == 0), stop=(i == HTa - 1))
                nc.vector.tensor_copy(vh_t[sc][:], ps[:, 0:D])

            # ---------------- attention pipeline ----------------
            def emit_qk_exp_mask(qb, kc):
                qsl = slice(qb * QB, (qb + 1) * QB)
                pms = []
                for pr in range(NPAIR):
                    s_ps = sps.tile([P, 2 * QB], F32, tag="s", name="s")
                    for hb in range(2):
                        rsl = slice(64 * hb, 64 * hb + 64)
                        nc.tensor.matmul(
                            s_ps[:, hb * QB:(hb + 1) * QB],
                            kh_t[pr][rsl, kc * P:(kc + 1) * P],
                            qh_t[pr][rsl, qsl],
                            start=True, stop=True)
                    p_t = pp.tile([P, 2 * QB], BF16, tag="p", name="p")
                    nc.scalar.activation(p_t[:], s_ps[:], Exp, scale=0.125)
                    pm = pmp.tile([P, 2 * QB], BF16, tag="pm", name="pm")
                    eng = nc.vector
                    if USE_GPSIMD_MASK and (pr == 1) and (kc % 2 == 1):
                        eng = nc.gpsimd
                    for hb in range(2):
                        eng.tensor_mul(
                            pm[:, hb * QB:(hb + 1) * QB],
                            p_t[:, hb * QB:(hb + 1) * QB],
                            mask_t[kc][:, qsl])
                    pms.append(pm)
                return pms

            def emit_av_rs(qb, kc, pms):
                for pr in range(NPAIR):
                    for hb in range(2):
                        h = 2 * pr + hb
                        nc.tensor.matmul(
                            x_ps[pr][64 * hb:64 * hb + 64, :],
                            vh_t[kc][:, h * DK:(h + 1) * DK],
                            pms[pr][:, hb * QB:(hb + 1) * QB],
                            start=(kc == 0), stop=(kc == SH - 1),
                            skip_group_check=True)
                for pr in range(NPAIR):
                    for hb in range(2):
                        h = 2 * pr + hb
                        nc.tensor.matmul(
                            rs_ps[32 * h:32 * h + 1, :],
                            ones_t[:, 0:1],
                            pms[pr][:, hb * QB:(hb + 1) * QB],
                            start=(kc == 0), stop=(kc == SH - 1),
                            skip_group_check=True,
                            tile_position=(0, 32 * h))

            def normalize(qb):
                """r = 1/rowsum, broadcast via DRAM, xn = x_ps * r."""
                qsl = slice(qb * QB, (qb + 1) * QB)
                r32 = rfp.tile([P, QB], F32, tag="r32", name="r32")
                r16 = rfp.tile([P, QB], BF16, tag="r16", name="r16")
                nc.vector.reciprocal_approx_fast(out=r32[:], in_=rs_ps[:])
                nc.vector.tensor_copy(r16[:], r32[:])
                rows = r16[:].rearrange("(g p) q -> g p q", p=32)[:, 0:1, :]
                nc.sync.dma_start(
                    rb_d[qb * NH:(qb + 1) * NH, :].rearrange(
                        "(g o) q -> g o q", o=1),
                    rows)
                for pr in range(NPAIR):
                    rb = rbp.tile([P, QB], BF16, tag="rb", name="rb")
                    for hb in range(2):
                        row = qb * NH + 2 * pr + hb
                        nc.sync.dma_start(
                            rb[64 * hb:64 * hb + 64, :],
                            rb_d[row:row + 1, :].broadcast_to([64, QB]))
                    nc.vector.tensor_mul(
                        xn_t[pr][:, qsl], x_ps[pr][:], rb[:])

            def oproj_chunk(qb, hc, pool):
                qsl = slice(qb * QB, (qb + 1) * QB)
                if pool is yps:
                    y_ps = pool.tile([P, QB], F32, tag="y", name="y")
                else:
                    y_ps = pool.tile([P, 2 * QB], F32, tag="s",
                                     name="y")[:, 0:QB]
                for pr in range(NPAIR):
                    nc.tensor.matmul(
                        y_ps[:],
                        wo_t[pr][:, hc * P:(hc + 1) * P],
                        xn_t[pr][:, qsl],
                        start=(pr == 0), stop=(pr == NPAIR - 1))
                y_sb = ysb.tile([P, QB], BF16, tag="ysb", name="ysb")
                nc.vector.tensor_copy(y_sb[:], y_ps[:])
                (nc.sync if hc % 2 == 0 else nc.gpsimd).dma_start(
                    y_d[hc * P:(hc + 1) * P, qsl], y_sb[:])

            # ---------------- emission schedule ----------------
            proj_chunk(kT_c[0], wk_t, kh_t, 0, nc.scalar)
            proj_chunk(qT_c[0], wq_t, qh_t, 0, nc.scalar)

            # (qb, kc) -> fused PE filler emitted just before that QK;
            # proj chunks split per-dc to halve the PE spike.
            filler = {
                (0, 1): lambda: late_loads(),
                (0, 2): lambda: proj_chunk(kT_c[1], wk_t, kh_t, 1, nc.vector,
                                           (0,)),
                (0, 3): lambda: proj_chunk(kT_c[1], wk_t, kh_t, 1, nc.vector,
                                           (1,)),
                (0, 6): lambda: proj_chunk(kT_c[2], wk_t, kh_t, 2, nc.vector,
                                           (0,)),
                (0, 7): lambda: proj_chunk(kT_c[2], wk_t, kh_t, 2, nc.vector,
                                           (1,)),
                (0, 10): lambda: proj_chunk(kT_c[3], wk_t, kh_t, 3, nc.vector,
                                            (0,)),
                (0, 11): lambda: proj_chunk(kT_c[3], wk_t, kh_t, 3, nc.vector,
                                            (1,)),
                (0, 13): lambda: proj_chunk(qT_c[1], wq_t, qh_t, 1, nc.vector,
                                            (0,)),
                (0, 14): lambda: proj_chunk(qT_c[1], wq_t, qh_t, 1, nc.vector,
                                            (1,)),
                (1, 2): lambda: proj_chunk(qT_c[2], wq_t, qh_t, 2, nc.vector,
                                           (0,)),
                (1, 3): lambda: proj_chunk(qT_c[2], wq_t, qh_t, 2, nc.vector,
                                           (1,)),
                (1, 8): lambda: proj_chunk(qT_c[3], wq_t, qh_t, 3, nc.vector,
                                           (0,)),
                (1, 9): lambda: proj_chunk(qT_c[3], wq_t, qh_t, 3, nc.vector,
                                           (1,)),
            }

            pending = deque()
            oproj_q = deque()

            def pop_pending():
                qb0, kc0, pms0 = pending.popleft()
                emit_av_rs(qb0, kc0, pms0)
                if kc0 == SH - 1:
                    normalize(qb0)
                    for hc in range(HT):
                        oproj_q.append((qb0, hc))

            t = 0
            for qb in range(QBn):
                for kc in range(SH):
                    if (qb, kc) in filler:
                        filler[(qb, kc)]()
                    pms = emit_qk_exp_mask(qb, kc)
                    if qb == 0:
                        vh_proj(kc)
                    pending.append((qb, kc, pms))
                    if len(pending) > LAG:
                        pop_pending()
                    if t % OPROJ_EVERY == 1 and oproj_q:
                        oproj_chunk(*oproj_q.popleft(), yps)
                    t += 1
            while pending:
                pop_pending()
            while oproj_q:
                oproj_chunk(*oproj_q.popleft(), sps)

    nc.compile()
    return nc


def make_in_maps(q, k, v, mask, Wq, bq, Wk, bk, Wv, bv, Wo,
                 n_cores=8, NH=4, DK=64, aug=False):
    bf = ml_dtypes.bfloat16
    B, S, HID = q.shape
    D = NH * DK
    n_hg = n_cores // B

    def with_aug(xT, bias_row):
        pad = np.zeros((P, xT.shape[1]), xT.dtype)
        pad[0, :] = bias_row
        return np.concatenate([xT, pad], axis=0)

    per_batch = {}
    for b in range(B):
        qT = np.ascontiguousarray(q[b].T).astype(bf)
        kT = np.ascontiguousarray(k[b].T).astype(bf)
        vT = np.ascontiguousarray(v[b].T).astype(bf)
        if aug:
            one = np.ones((S,), np.float32).astype(bf)
            qT, kT, vT = with_aug(qT, one), with_aug(kT, one), with_aug(vT, one)
        per_batch[b] = (qT, kT, vT,
                        np.ascontiguousarray(mask[b, 0].T != 0).astype(bf))

    in_maps = []
    for core in range(n_cores):
        b, hg = divmod(core, n_hg)
        hsl = slice(hg * D, (hg + 1) * D)
        wq = Wq[:, hsl].astype(bf)
        wk = Wk[:, hsl].astype(bf)
        wv = Wv[:, hsl].astype(bf)
        if aug:
            wq = with_aug(wq, bq[hsl].astype(bf))
            wk = with_aug(wk, bk[hsl].astype(bf))
            wv = with_aug(wv, bv[hsl].astype(bf))
        qT, kT, vT, mT = per_batch[b]
        in_maps.append(dict(
            qT=qT, kT=kT, vT=vT, maskT=mT,
            wq=np.ascontiguousarray(wq), wk=np.ascontiguousarray(wk),
            wv=np.ascontiguousarray(wv),
            wo=np.ascontiguousarray(Wo[hsl, :]).astype(bf),
        ))
    return in_maps


def combine_outputs(results, B, S, HID, bo, n_cores=8):
    n_hg = n_cores // B
    out = np.zeros((B, S, HID), np.float32)
    for core in range(n_cores):
        b = core // n_hg
        out[b] += results[core]["y"].astype(np.float32).T
    return out + bo.astype(np.float32)


def run_mha(q, k, v, mask, Wq, bq, Wk, bk, Wv, bv, Wo, bo, trace=False):
    from concourse.bass_utils import run_bass_kernel_spmd

    B, S, HID = q.shape
    n_cores = 8
    aug = bool(np.any(bq) or np.any(bk) or np.any(bv))
    key = (S, HID, aug)
    if key not in _PROGRAM_CACHE:
        _PROGRAM_CACHE[key] = build_mha_program(S=S, HID=HID, aug=aug)
    nc = _PROGRAM_CACHE[key]
    in_maps = make_in_maps(q, k, v, mask, Wq, bq, Wk, bk, Wv, bv, Wo,
                           n_cores=n_cores, aug=aug)
    res = run_bass_kernel_spmd(nc, in_maps, list(range(n_cores)), trace=trace)
    out = combine_outputs(res.results, B, S, HID, bo, n_cores=n_cores)
    return out, res


def kernel(q, k, v, mask, Wq, bq, Wk, bk, Wv, bv, Wo, bo):
    q = np.asarray(q, np.float32)
    k = np.asarray(k, np.float32)
    v = np.asarray(v, np.float32)
    mask = np.asarray(mask)
    out, _ = run_mha(q, k, v, mask,
                     np.asarray(Wq, np.float32), np.asarray(bq, np.float32),
                     np.asarray(Wk, np.float32), np.asarray(bk, np.float32),
                     np.asarray(Wv, np.float32), np.asarray(bv, np.float32),
                     np.asarray(Wo, np.float32), np.asarray(bo, np.float32))
    return out


# revision 29
# speedup vs baseline: 1.1225x; 1.0009x over previous
"""TRN2 Bass kernel: 16-head MHA (B=2, S=2048, H=1024) sharded over 8 NeuronCores.

Sharding: data-parallel over batch (2) x tensor-parallel over head groups
(4 groups of 4 heads). Each core computes its 4 heads' attention for its batch
and a partial output projection; the host sums the 4 partials per batch,
transposes, and adds the output bias.

v3: fully fused single-phase pipeline.
  - QK^T head pairs issued as concurrent row-tiles ((0,0)/(64,0)); AV pairs as
    col-tiles ((0,0)/(0,64)); softmax denominators from a 4-way col-tiled pass
    of M=1 ones-matmuls accumulating in a dedicated PSUM bank.
  - exp() is one [128, 1024] ACTIVATE per (qb, kc, pair); 1/rowsum via
    reciprocal_approx_fast; partition-broadcast via a bf16 DRAM round trip;
    normalization multiplies PSUM x directly (no intermediate xu).
  - Projections are fused into the attention stream: inputs arrive as
    column-chunk DMAs on three queues; Q/K projection chunks and per-kc V
    projections run as PE filler inside q-block 0/1/2, sharing one PSUM bank
    ring with the output projection.
  - Output projection chunks trail one q-block behind; y is written bf16
    (host sums the 4 partials per batch in fp32).
"""

import sys

sys.path.insert(0, "/opt/trn_rl_repo")

from collections import deque
from contextlib import ExitStack

import numpy as np
import ml_dtypes

import concourse.tile as tile
from concourse import bacc, mybir

BF16 = mybir.dt.bfloat16
F32 = mybir.dt.float32
P = 128

LAG = 4            # kc-instances by which AV/rowsum matmuls trail QK/exp/mask
USE_GPSIMD_MASK = True   # offload 1/4 of mask multiplies to GPSIMD
OPROJ_EVERY = 2    # pop one oproj chunk every N kc-instances

_PROGRAM_CACHE = {}


def build_mha_program(S=2048, HID=1024, NH=4, DK=64, QB=512, aug=False):
    """Build + compile the per-core SPMD Bass program."""
    D = NH * DK
    assert NH == 4 and DK == 64
    SH = S // P                 # 16 key blocks
    HT = HID // P               # 8 hidden blocks
    HTa = HT + (1 if aug else 0)
    QBn = S // QB               # 4 q-blocks
    NPAIR = NH // 2             # 2 head pairs
    CH = S // QB                # 4 input column chunks (same size as QB)

    nc = bacc.Bacc("TRN2", target_bir_lowering=False, debug=False)

    qT_d = nc.dram_tensor("qT", [HTa * P, S], BF16, kind="ExternalInput").ap()
    kT_d = nc.dram_tensor("kT", [HTa * P, S], BF16, kind="ExternalInput").ap()
    vT_d = nc.dram_tensor("vT", [HTa * P, S], BF16, kind="ExternalInput").ap()
    maskT_d = nc.dram_tensor("maskT", [S, S], BF16, kind="ExternalInput").ap()
    wq_d = nc.dram_tensor("wq", [HTa * P, D], BF16, kind="ExternalInput").ap()
    wk_d = nc.dram_tensor("wk", [HTa * P, D], BF16, kind="ExternalInput").ap()
    wv_d = nc.dram_tensor("wv", [HTa * P, D], BF16, kind="ExternalInput").ap()
    wo_d = nc.dram_tensor("wo", [D, HID], BF16, kind="ExternalInput").ap()
    y_d = nc.dram_tensor("y", [HID, S], BF16, kind="ExternalOutput").ap()
    # DRAM bounce buffer for partition-broadcasting the softmax reciprocals
    rb_d = nc.dram_tensor("r_bounce", [NH * QBn, QB], BF16).ap()

    Exp = mybir.ActivationFunctionType.Exp

    with tile.TileContext(nc) as tc:
        with ExitStack() as ctx:
            persist = ctx.enter_context(tc.tile_pool(name="persist", bufs=1))
            wpool = ctx.enter_context(tc.tile_pool(name="wpool", bufs=1))
            inq = ctx.enter_context(tc.tile_pool(name="inq", bufs=2))
            mp = ctx.enter_context(tc.tile_pool(name="mask", bufs=1))
            pp = ctx.enter_context(tc.tile_pool(name="pexp", bufs=2))
            pmp = ctx.enter_context(
                tc.tile_pool(name="pmask", bufs=2 * (LAG + 2)))
            rfp = ctx.enter_context(tc.tile_pool(name="rfp", bufs=2))
            rbp = ctx.enter_context(tc.tile_pool(name="rbp", bufs=2))
            ysb = ctx.enter_context(tc.tile_pool(name="ysb", bufs=2))
            sps = ctx.enter_context(
                tc.tile_pool(name="sps", bufs=2, space="PSUM"))
            xps = ctx.enter_context(
                tc.tile_pool(name="xps", bufs=1, space="PSUM"))
            rsps = ctx.enter_context(
                tc.tile_pool(name="rsps", bufs=1, space="PSUM"))
            yps = ctx.enter_context(
                tc.tile_pool(name="yps", bufs=1, space="PSUM"))

            qh_t = [persist.tile([P, S], BF16, tag=f"qh{d}", name=f"qh{d}")
                    for d in range(NPAIR)]
            kh_t = [persist.tile([P, S], BF16, tag=f"kh{d}", name=f"kh{d}")
                    for d in range(NPAIR)]
            vh_t = [persist.tile([P, D], BF16, tag=f"vh{s}", name=f"vh{s}")
                    for s in range(SH)]
            xn_t = [persist.tile([P, S], BF16, tag=f"xn{p}", name=f"xn{p}")
                    for p in range(NPAIR)]
            wo_t = [persist.tile([P, HID], BF16, tag=f"wo{p}", name=f"wo{p}")
                    for p in range(NPAIR)]
            ones_t = persist.tile([P, 4], BF16, tag="ones", name="ones")
            nc.vector.memset(ones_t[:], 1.0)

            wq_t = wpool.tile([P, HTa * D], BF16, tag="wq", name="wq")
            wk_t = wpool.tile([P, HTa * D], BF16, tag="wk", name="wk")
            wv_t = wpool.tile([P, HTa * D], BF16, tag="wv", name="wv")

            mask_t = [mp.tile([P, S], BF16, tag=f"m{i}", name=f"m{i}")
                      for i in range(SH)]

            x_ps = [xps.tile([P, QB], F32, tag=f"x{p}", name=f"x{p}")
                    for p in range(NPAIR)]
            rs_ps = rsps.tile([P, QB], F32, tag="rs", name="rs")

            def wload(eng, dst_t, src_d):
                dst = dst_t[:].rearrange("p (i s) -> p i s", s=D)
                src = src_d[:, :].rearrange("(i p) s -> p i s", p=P)
                eng.dma_start(dst, src)

            def chunk_load(eng, tag, src_d, c):
                """Column chunk c (QB cols) of all HTa row-blocks."""
                t = inq.tile([P, HTa * QB], BF16, tag=tag, name=f"{tag}{c}")
                dst = t[:].rearrange("p (i s) -> p i s", s=QB)
                src = src_d[:, c * QB:(c + 1) * QB].rearrange(
                    "(i p) s -> p i s", p=P)
                eng.dma_start(dst, src)
                return t

            # -------- DMA schedule (3 queues: sync / scalar / gpsimd) -------
            # Head of pipeline: kT0 alone on scalar (gates kh-c0 proj),
            # wq+qT0 on sync (gates qh-c0 proj), wk/wv early on gpsimd.
            qT_c, kT_c, vT_c = {}, {}, {}
            kT_c[0] = chunk_load(nc.scalar, "kT", kT_d, 0)
            wload(nc.sync, wq_t, wq_d)
            qT_c[0] = chunk_load(nc.sync, "qT", qT_d, 0)
            wload(nc.gpsimd, wk_t, wk_d)
            wload(nc.gpsimd, wv_t, wv_d)
            kT_c[1] = chunk_load(nc.scalar, "kT", kT_d, 1)
            nc.sync.dma_start(mask_t[0][:], maskT_d[0:P, :])
            vT_c[0] = chunk_load(nc.gpsimd, "vT", vT_d, 0)
            nc.sync.dma_start(mask_t[1][:], maskT_d[P:2 * P, :])
            qT_c[1] = chunk_load(nc.scalar, "qT", qT_d, 1)
            vT_c[1] = chunk_load(nc.gpsimd, "vT", vT_d, 1)
            nc.gpsimd.dma_start(wo_t[0][:], wo_d[0:P, :])
            nc.gpsimd.dma_start(wo_t[1][:], wo_d[P:2 * P, :])
            for i in range(2, 8):
                (nc.gpsimd if i % 2 == 0 else nc.sync).dma_start(
                    mask_t[i][:], maskT_d[i * P:(i + 1) * P, :])

            def mload(eng, i):
                eng.dma_start(mask_t[i][:], maskT_d[i * P:(i + 1) * P, :])

            def late_loads():
                """Issued mid-stream on sync/gpsimd (never scalar: its queue
                carries the exp stream). Later tranches are staggered so the
                latency-critical rb/y DMAs interleave between them."""
                kT_c[2] = chunk_load(nc.gpsimd, "kT", kT_d, 2)
                vT_c[2] = chunk_load(nc.sync, "vT", vT_d, 2)
                kT_c[3] = chunk_load(nc.gpsimd, "kT", kT_d, 3)
                qT_c[2] = chunk_load(nc.scalar, "qT", qT_d, 2)
                vT_c[3] = chunk_load(nc.gpsimd, "vT", vT_d, 3)
                qT_c[3] = chunk_load(nc.scalar, "qT", qT_d, 3)
                for i in range(8, SH):
                    mload(nc.sync if i % 2 == 0 else nc.gpsimd, i)

            # ---------------- fused projection helpers ----------------
            def proj_chunk(src_c, w_t, dst, qc, eng, dcs=(0, 1)):
                """dst[:, qc chunk] = (w.T @ src) for the given row halves."""
                for dc in dcs:
                    ps = yps.tile([P, QB], F32, tag="y", name=f"pj{qc}{dc}")
                    for i in range(HTa):
                        nc.tensor.matmul(
                            ps[:],
                            w_t[:, i * D + dc * P:i * D + (dc + 1) * P],
                            src_c[:, i * QB:(i + 1) * QB],
                            start=(i == 0), stop=(i == HTa - 1))
                    dst_ap = dst[dc][:, qc * QB:(qc + 1) * QB]
                    if eng is nc.scalar:
                        eng.copy(dst_ap, ps[:])
                    else:
                        eng.tensor_copy(dst_ap, ps[:])

            def vh_proj(sc):
                """vh[sc] = vT[:, sc block].T @ wv  (one [P, D] tile)."""
                c = sc // (QB // P)
                off = (sc % (QB // P)) * P
                ps = yps.tile([P, QB], F32, tag="y", name=f"vj{sc}")
                for i in range(HTa):
                    nc.tensor.matmul(
                        ps[:, 0:D],
                        vT_c[c][:, i * QB + off:i * QB + off + P],
                        wv_t[:, i * D:(i + 1) * D],
                        start=(i == 0), stop=(i == HTa - 1))
                nc.vector.tensor_copy(vh_t[sc][:], ps[:, 0:D])

            # ---------------- attention pipeline ----------------
            def emit_qk_exp_mask(qb, kc):
                qsl = slice(qb * QB, (qb + 1) * QB)
                pms = []
                for pr in range(NPAIR):
                    s_ps = sps.tile([P, 2 * QB], F32, tag="s", name="s")
                    for hb in range(2):
                        rsl = slice(64 * hb, 64 * hb + 64)
                        nc.tensor.matmul(
                            s_ps[:, hb * QB:(hb + 1) * QB],
                            kh_t[pr][rsl, kc * P:(kc + 1) * P],
                            qh_t[pr][rsl, qsl],
                            start=True, stop=True)
                    p_t = pp.tile([P, 2 * QB], BF16, tag="p", name="p")
                    nc.scalar.activation(p_t[:], s_ps[:], Exp, scale=0.125)
                    pm = pmp.tile([P, 2 * QB], BF16, tag="pm", name="pm")
                    eng = nc.vector
                    if USE_GPSIMD_MASK and (pr == 1) and (kc % 2 == 1):
                        eng = nc.gpsimd
                    for hb in range(2):
                        eng.tensor_mul(
                            pm[:, hb * QB:(hb + 1) * QB],
                            p_t[:, hb * QB:(hb + 1) * QB],
                            mask_t[kc][:, qsl])
                    pms.append(pm)
                return pms

            def emit_av_rs(qb, kc, pms):
                for pr in range(NPAIR):
                    for hb in range(2):
                        h = 2 * pr + hb
                        nc.tensor.matmul(
                            x_ps[pr][64 * hb:64 * hb + 64, :],
                            vh_t[kc][:, h * DK:(h + 1) * DK],
                            pms[pr][:, hb * QB:(hb + 1) * QB],
                            start=(kc == 0), stop=(kc == SH - 1),
                            skip_group_check=True)
                for pr in range(NPAIR):
                    for hb in range(2):
                        h = 2 * pr + hb
                        nc.tensor.matmul(
                            rs_ps[32 * h:32 * h + 1, :],
                            ones_t[:, 0:1],
                            pms[pr][:, hb * QB:(hb + 1) * QB],
                            start=(kc == 0), stop=(kc == SH - 1),
                            skip_group_check=True,
                            tile_position=(0, 32 * h))

            def normalize(qb):
                """r = 1/rowsum, broadcast via DRAM, xn = x_ps * r."""
                qsl = slice(qb * QB, (qb + 1) * QB)
                r32 = rfp.tile([P, QB], F32, tag="r32", name="r32")
                r16 = rfp.tile([P, QB], BF16, tag="r16", name="r16")
                nc.vector.reciprocal_approx_fast(out=r32[:], in_=rs_ps[:])
                nc.vector.tensor_copy(r16[:], r32[:])
                rows = r16[:].rearrange("(g p) q -> g p q", p=32)[:, 0:1, :]
                nc.sync.dma_start(
                    rb_d[qb * NH:(qb + 1) * NH, :].rearrange(
                        "(g o) q -> g o q", o=1),
                    rows)
                for pr in range(NPAIR):
                    rb = rbp.tile([P, QB], BF16, tag="rb", name="rb")
                    for hb in range(2):
                        row = qb * NH + 2 * pr + hb
                        nc.sync.dma_start(
                            rb[64 * hb:64 * hb + 64, :],
                            rb_d[row:row + 1, :].broadcast_to([64, QB]))
                    nc.vector.tensor_mul(
                        xn_t[pr][:, qsl], x_ps[pr][:], rb[:])

            def oproj_chunk(qb, hc, pool):
                qsl = slice(qb * QB, (qb + 1) * QB)
                if pool is yps:
                    y_ps = pool.tile([P, QB], F32, tag="y", name="y")
                else:
                    y_ps = pool.tile([P, 2 * QB], F32, tag="s",
                                     name="y")[:, 0:QB]
                for pr in range(NPAIR):
                    nc.tensor.matmul(
                        y_ps[:],
                        wo_t[pr][:, hc * P:(hc + 1) * P],
                        xn_t[pr][:, qsl],
                        start=(pr == 0), stop=(pr == NPAIR - 1))
                y_sb = ysb.tile([P, QB], BF16, tag="ysb", name="ysb")
                nc.vector.tensor_copy(y_sb[:], y_ps[:])
                (nc.sync if hc % 2 == 0 else nc.gpsimd).dma_start(
                    y_d[hc * P:(hc + 1) * P, qsl], y_sb[:])

            # ---------------- emission schedule ----------------
            proj_chunk(kT_c[0], wk_t, kh_t, 0, nc.scalar)
            proj_chunk(qT_c[0], wq_t, qh_t, 0, nc.scalar)

            # (qb, kc) -> fused PE filler emitted just before that QK;
            # proj chunks split per-dc to halve the PE spike.
            filler = {
                (0, 1): lambda: late_loads(),
                (0, 2): lambda: proj_chunk(kT_c[1], wk_t, kh_t, 1, nc.vector,
                                           (0,)),
                (0, 3): lambda: proj_chunk(kT_c[1], wk_t, kh_t, 1, nc.vector,
                                           (1,)),
                (0, 6): lambda: proj_chunk(kT_c[2], wk_t, kh_t, 2, nc.vector,
                                           (0,)),
                (0, 7): lambda: proj_chunk(kT_c[2], wk_t, kh_t, 2, nc.vector,
                                           (1,)),
                (0, 10): lambda: proj_chunk(kT_c[3], wk_t, kh_t, 3, nc.vector,
                                            (0,)),
                (0, 11): lambda: proj_chunk(kT_c[3], wk_t, kh_t, 3, nc.vector,
                                            (1,)),
                (0, 13): lambda: proj_chunk(qT_c[1], wq_t, qh_t, 1, nc.vector,
                                            (0,)),
                (0, 14): lambda: proj_chunk(qT_c[1], wq_t, qh_t, 1, nc.vector,
                                            (1,)),
                (1, 2): lambda: proj_chunk(qT_c[2], wq_t, qh_t, 2, nc.vector,
                                           (0,)),
                (1, 3): lambda: proj_chunk(qT_c[2], wq_t, qh_t, 2, nc.vector,
                                           (1,)),
                (1, 8): lambda: proj_chunk(qT_c[3], wq_t, qh_t, 3, nc.vector,
                                           (0,)),
                (1, 9): lambda: proj_chunk(qT_c[3], wq_t, qh_t, 3, nc.vector,
                                           (1,)),
            }

            pending = deque()
            oproj_q = deque()

            def pop_pending():
                qb0, kc0, pms0 = pending.popleft()
                emit_av_rs(qb0, kc0, pms0)
                if kc0 == SH - 1:
                    normalize(qb0)
                    for hc in range(HT):
                        oproj_q.append((qb0, hc))

            t = 0
            for qb in range(QBn):
                for kc in range(SH):
                    if (qb, kc) in filler:
                        filler[(qb, kc)]()
                    pms = emit_qk_exp_mask(qb, kc)
                    if qb == 0:
                        vh_proj(kc)
                    pending.append((qb, kc, pms))
                    if len(pending) > LAG:
                        pop_pending()
                    if t % OPROJ_EVERY == 1 and oproj_q:
                        oproj_chunk(*oproj_q.popleft(), yps)
                    t += 1
            while pending:
                pop_pending()
            while oproj_q:
                oproj_chunk(*oproj_q.popleft(), sps)

    nc.compile()
    return nc


def make_in_maps(q, k, v, mask, Wq, bq, Wk, bk, Wv, bv, Wo,
                 n_cores=8, NH=4, DK=64, aug=False):
    bf = ml_dtypes.bfloat16
    B, S, HID = q.shape
    D = NH * DK
    n_hg = n_cores // B

    def with_aug(xT, bias_row):
        pad = np.zeros((P, xT.shape[1]), xT.dtype)
        pad[0, :] = bias_row
        return np.concatenate([xT, pad], axis=0)

    per_batch = {}
    for b in range(B):
        qT = np.ascontiguousarray(q[b].T).astype(bf)
        kT = np.ascontiguousarray(k[b].T).astype(bf)
        vT = np.ascontiguousarray(v[b].T).astype(bf)
        if aug:
            one = np.ones((S,), np.float32).astype(bf)
            qT, kT, vT = with_aug(qT, one), with_aug(kT, one), with_aug(vT, one)
        per_batch[b] = (qT, kT, vT,
                        np.ascontiguousarray(mask[b, 0].T != 0).astype(bf))

    in_maps = []
    for core in range(n_cores):
        b, hg = divmod(core, n_hg)
        hsl = slice(hg * D, (hg + 1) * D)
        wq = Wq[:, hsl].astype(bf)
        wk = Wk[:, hsl].astype(bf)
        wv = Wv[:, hsl].astype(bf)
        if aug:
            wq = with_aug(wq, bq[hsl].astype(bf))
            wk = with_aug(wk, bk[hsl].astype(bf))
            wv = with_aug(wv, bv[hsl].astype(bf))
        qT, kT, vT, mT = per_batch[b]
        in_maps.append(dict(
            qT=qT, kT=kT, vT=vT, maskT=mT,
            wq=np.ascontiguousarray(wq), wk=np.ascontiguousarray(wk),
            wv=np.ascontiguousarray(wv),
            wo=np.ascontiguousarray(Wo[hsl, :]).astype(bf),
        ))
    return in_maps


def combine_outputs(results, B, S, HID, bo, n_cores=8):
    n_hg = n_cores // B
    out = np.zeros((B, S, HID), np.float32)
    for core in range(n_cores):
        b = core // n_hg
        out[b] += results[core]["y"].astype(np.float32).T
    return out + bo.astype(np.float32)


def run_mha(q, k, v, mask, Wq, bq, Wk, bk, Wv, bv, Wo, bo, trace=False):
    from concourse.bass_utils import run_bass_kernel_spmd

    B, S, HID = q.shape
    n_cores = 8
    aug = bool(np.any(bq) or np.any(bk) or np.any(bv))
    key = (S, HID, aug)
    if key not in _PROGRAM_CACHE:
        _PROGRAM_CACHE[key] = build_mha_program(S=S, HID=HID, aug=aug)
    nc = _PROGRAM_CACHE[key]
    in_maps = make_in_maps(q, k, v, mask, Wq, bq, Wk, bk, Wv, bv, Wo,
                           n_cores=n_cores, aug=aug)
    res = run_bass_kernel_spmd(nc, in_maps, list(range(n_cores)), trace=trace)
    out = combine_outputs(res.results, B, S, HID, bo, n_cores=n_cores)
    return out, res


def kernel(q, k, v, mask, Wq, bq, Wk, bk, Wv, bv, Wo, bo):
    q = np.asarray(q, np.float32)
    k = np.asarray(k, np.float32)
    v = np.asarray(v, np.float32)
    mask = np.asarray(mask)
    out, _ = run_mha(q, k, v, mask,
                     np.asarray(Wq, np.float32), np.asarray(bq, np.float32),
                     np.asarray(Wk, np.float32), np.asarray(bk, np.float32),
                     np.asarray(Wv, np.float32), np.asarray(bv, np.float32),
                     np.asarray(Wo, np.float32), np.asarray(bo, np.float32))
    return out


# revision 34
# speedup vs baseline: 1.1364x; 1.0124x over previous
"""TRN2 Bass kernel: 16-head MHA (B=2, S=2048, H=1024) sharded over 8 NeuronCores.

Sharding: data-parallel over batch (2) x tensor-parallel over head groups
(4 groups of 4 heads). Each core computes its 4 heads' attention for its batch
and a partial output projection; the host sums the 4 partials per batch,
transposes, and adds the output bias.

v3: fully fused single-phase pipeline.
  - QK^T head pairs issued as concurrent row-tiles ((0,0)/(64,0)); AV pairs as
    col-tiles ((0,0)/(0,64)); softmax denominators from a 4-way col-tiled pass
    of M=1 ones-matmuls accumulating in a dedicated PSUM bank.
  - exp() is one [128, 1024] ACTIVATE per (qb, kc, pair); 1/rowsum via
    reciprocal_approx_fast; partition-broadcast via a bf16 DRAM round trip;
    normalization multiplies PSUM x directly (no intermediate xu).
  - Projections are fused into the attention stream: inputs arrive as
    column-chunk DMAs on three queues; Q/K projection chunks and per-kc V
    projections run as PE filler inside q-block 0/1/2, sharing one PSUM bank
    ring with the output projection.
  - Output projection chunks trail one q-block behind; y is written bf16
    (host sums the 4 partials per batch in fp32).
"""

import sys

sys.path.insert(0, "/opt/trn_rl_repo")

from collections import deque
from contextlib import ExitStack

import numpy as np
import ml_dtypes

import concourse.tile as tile
from concourse import bacc, mybir

BF16 = mybir.dt.bfloat16
F32 = mybir.dt.float32
P = 128

LAG = 4            # kc-instances by which AV/rowsum matmuls trail QK/exp/mask
USE_GPSIMD_MASK = True   # offload 1/4 of mask multiplies to GPSIMD
OPROJ_EVERY = 2    # pop one oproj chunk every N kc-instances

_PROGRAM_CACHE = {}


def build_mha_program(S=2048, HID=1024, NH=4, DK=64, QB=512, aug=False):
    """Build + compile the per-core SPMD Bass program."""
    D = NH * DK
    assert NH == 4 and DK == 64
    SH = S // P                 # 16 key blocks
    HT = HID // P               # 8 hidden blocks
    HTa = HT + (1 if aug else 0)
    QBn = S // QB               # 4 q-blocks
    NPAIR = NH // 2             # 2 head pairs
    CH = S // QB                # 4 input column chunks (same size as QB)

    nc = bacc.Bacc("TRN2", target_bir_lowering=False, debug=False)

    qT_d = nc.dram_tensor("qT", [HTa * P, S], BF16, kind="ExternalInput").ap()
    kT_d = nc.dram_tensor("kT", [HTa * P, S], BF16, kind="ExternalInput").ap()
    vT_d = nc.dram_tensor("vT", [HTa * P, S], BF16, kind="ExternalInput").ap()
    maskT_d = nc.dram_tensor("maskT", [S, S], BF16, kind="ExternalInput").ap()
    wq_d = nc.dram_tensor("wq", [HTa * P, D], BF16, kind="ExternalInput").ap()
    wk_d = nc.dram_tensor("wk", [HTa * P, D], BF16, kind="ExternalInput").ap()
    wv_d = nc.dram_tensor("wv", [HTa * P, D], BF16, kind="ExternalInput").ap()
    wo_d = nc.dram_tensor("wo", [D, HID], BF16, kind="ExternalInput").ap()
    y_d = nc.dram_tensor("y", [HID, S], BF16, kind="ExternalOutput").ap()
    # DRAM bounce buffer for partition-broadcasting the softmax reciprocals
    rb_d = nc.dram_tensor("r_bounce", [NH * QBn, QB], BF16).ap()

    Exp = mybir.ActivationFunctionType.Exp

    with tile.TileContext(nc) as tc:
        with ExitStack() as ctx:
            persist = ctx.enter_context(tc.tile_pool(name="persist", bufs=1))
            wpool = ctx.enter_context(tc.tile_pool(name="wpool", bufs=1))
            inq = ctx.enter_context(tc.tile_pool(name="inq", bufs=2))
            mp = ctx.enter_context(tc.tile_pool(name="mask", bufs=1))
            pp = ctx.enter_context(tc.tile_pool(name="pexp", bufs=2))
            pmp = ctx.enter_context(
                tc.tile_pool(name="pmask", bufs=2 * (LAG + 2)))
            rfp = ctx.enter_context(tc.tile_pool(name="rfp", bufs=2))
            rbp = ctx.enter_context(tc.tile_pool(name="rbp", bufs=2))
            ysb = ctx.enter_context(tc.tile_pool(name="ysb", bufs=2))
            sps = ctx.enter_context(
                tc.tile_pool(name="sps", bufs=2, space="PSUM"))
            xps = ctx.enter_context(
                tc.tile_pool(name="xps", bufs=1, space="PSUM"))
            rsps = ctx.enter_context(
                tc.tile_pool(name="rsps", bufs=1, space="PSUM"))
            yps = ctx.enter_context(
                tc.tile_pool(name="yps", bufs=1, space="PSUM"))

            qh_t = [persist.tile([P, S], BF16, tag=f"qh{d}", name=f"qh{d}")
                    for d in range(NPAIR)]
            kh_t = [persist.tile([P, S], BF16, tag=f"kh{d}", name=f"kh{d}")
                    for d in range(NPAIR)]
            vh_t = [persist.tile([P, D], BF16, tag=f"vh{s}", name=f"vh{s}")
                    for s in range(SH)]
            xn_t = [persist.tile([P, S], BF16, tag=f"xn{p}", name=f"xn{p}")
                    for p in range(NPAIR)]
            wo_t = [persist.tile([P, HID], BF16, tag=f"wo{p}", name=f"wo{p}")
                    for p in range(NPAIR)]
            ones_t = persist.tile([P, 4], BF16, tag="ones", name="ones")
            nc.vector.memset(ones_t[:], 1.0)

            wq_t = wpool.tile([P, HTa * D], BF16, tag="wq", name="wq")
            wk_t = wpool.tile([P, HTa * D], BF16, tag="wk", name="wk")
            wv_t = wpool.tile([P, HTa * D], BF16, tag="wv", name="wv")

            mask_t = [mp.tile([P, S], BF16, tag=f"m{i}", name=f"m{i}")
                      for i in range(SH)]

            x_ps = [xps.tile([P, QB], F32, tag=f"x{p}", name=f"x{p}")
                    for p in range(NPAIR)]
            rs_ps = rsps.tile([P, QB], F32, tag="rs", name="rs")

            def wload(eng, dst_t, src_d):
                dst = dst_t[:].rearrange("p (i s) -> p i s", s=D)
                src = src_d[:, :].rearrange("(i p) s -> p i s", p=P)
                eng.dma_start(dst, src)

            def chunk_load(eng, tag, src_d, c):
                """Column chunk c (QB cols) of all HTa row-blocks."""
                t = inq.tile([P, HTa * QB], BF16, tag=tag, name=f"{tag}{c}")
                dst = t[:].rearrange("p (i s) -> p i s", s=QB)
                src = src_d[:, c * QB:(c + 1) * QB].rearrange(
                    "(i p) s -> p i s", p=P)
                eng.dma_start(dst, src)
                return t

            # -------- DMA schedule (3 queues: sync / scalar / gpsimd) -------
            # Head of pipeline: kT0 alone on scalar (gates kh-c0 proj),
            # wq+qT0 on sync (gates qh-c0 proj), wk/wv early on gpsimd.
            qT_c, kT_c, vT_c = {}, {}, {}
            kT_c[0] = chunk_load(nc.scalar, "kT", kT_d, 0)
            wload(nc.sync, wq_t, wq_d)
            qT_c[0] = chunk_load(nc.sync, "qT", qT_d, 0)
            wload(nc.gpsimd, wk_t, wk_d)
            wload(nc.gpsimd, wv_t, wv_d)
            kT_c[1] = chunk_load(nc.scalar, "kT", kT_d, 1)
            nc.sync.dma_start(mask_t[0][:], maskT_d[0:P, :])
            vT_c[0] = chunk_load(nc.gpsimd, "vT", vT_d, 0)
            nc.sync.dma_start(mask_t[1][:], maskT_d[P:2 * P, :])
            qT_c[1] = chunk_load(nc.scalar, "qT", qT_d, 1)
            vT_c[1] = chunk_load(nc.gpsimd, "vT", vT_d, 1)
            nc.gpsimd.dma_start(wo_t[0][:], wo_d[0:P, :])
            nc.gpsimd.dma_start(wo_t[1][:], wo_d[P:2 * P, :])
            for i in range(2, 8):
                (nc.gpsimd if i % 2 == 0 else nc.sync).dma_start(
                    mask_t[i][:], maskT_d[i * P:(i + 1) * P, :])

            def mload(eng, i):
                eng.dma_start(mask_t[i][:], maskT_d[i * P:(i + 1) * P, :])

            def late_loads():
                """Issued mid-stream on sync/gpsimd (never scalar: its queue
                carries the exp stream). Later tranches are staggered so the
                latency-critical rb/y DMAs interleave between them."""
                kT_c[2] = chunk_load(nc.gpsimd, "kT", kT_d, 2)
                vT_c[2] = chunk_load(nc.sync, "vT", vT_d, 2)
                kT_c[3] = chunk_load(nc.gpsimd, "kT", kT_d, 3)
                qT_c[2] = chunk_load(nc.scalar, "qT", qT_d, 2)
                vT_c[3] = chunk_load(nc.gpsimd, "vT", vT_d, 3)
                qT_c[3] = chunk_load(nc.scalar, "qT", qT_d, 3)
                for i in range(8, SH):
                    mload(nc.sync if i % 2 == 0 else nc.gpsimd, i)

            # ---------------- fused projection helpers ----------------
            def proj_chunk(src_c, w_t, dst, qc, eng, dcs=(0, 1)):
                """dst[:, qc chunk] = (w.T @ src) for the given row halves."""
                for dc in dcs:
                    ps = yps.tile([P, QB], F32, tag="y", name=f"pj{qc}{dc}")
                    for i in range(HTa):
                        nc.tensor.matmul(
                            ps[:],
                            w_t[:, i * D + dc * P:i * D + (dc + 1) * P],
                            src_c[:, i * QB:(i + 1) * QB],
                            start=(i == 0), stop=(i == HTa - 1))
                    dst_ap = dst[dc][:, qc * QB:(qc + 1) * QB]
                    if eng is nc.scalar:
                        eng.copy(dst_ap, ps[:])
                    else:
                        eng.tensor_copy(dst_ap, ps[:])

            def vh_proj(sc):
                """vh[sc] = vT[:, sc block].T @ wv  (one [P, D] tile)."""
                c = sc // (QB // P)
                off = (sc % (QB // P)) * P
                ps = yps.tile([P, QB], F32, tag="y", name=f"vj{sc}")
                for i in range(HTa):
                    nc.tensor.matmul(
                        ps[:, 0:D],
                        vT_c[c][:, i * QB + off:i * QB + off + P],
                        wv_t[:, i * D:(i + 1) * D],
                        start=(i == 0), stop=(i == HTa - 1))
                nc.vector.tensor_copy(vh_t[sc][:], ps[:, 0:D])

            # ---------------- attention pipeline ----------------
            def emit_qk_exp_mask(qb, kc):
                qsl = slice(qb * QB, (qb + 1) * QB)
                pms = []
                for pr in range(NPAIR):
                    s_ps = sps.tile([P, 2 * QB], F32, tag="s", name="s")
                    for hb in range(2):
                        rsl = slice(64 * hb, 64 * hb + 64)
                        nc.tensor.matmul(
                            s_ps[:, hb * QB:(hb + 1) * QB],
                            kh_t[pr][rsl, kc * P:(kc + 1) * P],
                            qh_t[pr][rsl, qsl],
                            start=True, stop=True)
                    p_t = pp.tile([P, 2 * QB], BF16, tag="p", name="p")
                    nc.scalar.activation(p_t[:], s_ps[:], Exp, scale=0.125)
                    pm = pmp.tile([P, 2 * QB], BF16, tag="pm", name="pm")
                    eng = nc.vector
                    if USE_GPSIMD_MASK and (pr == 1) and (kc % 2 == 1):
                        eng = nc.gpsimd
                    for hb in range(2):
                        eng.tensor_mul(
                            pm[:, hb * QB:(hb + 1) * QB],
                            p_t[:, hb * QB:(hb + 1) * QB],
                            mask_t[kc][:, qsl])
                    pms.append(pm)
                return pms

            def emit_av_rs(qb, kc, pms):
                for pr in range(NPAIR):
                    for hb in range(2):
                        h = 2 * pr + hb
                        nc.tensor.matmul(
                            x_ps[pr][64 * hb:64 * hb + 64, :],
                            vh_t[kc][:, h * DK:(h + 1) * DK],
                            pms[pr][:, hb * QB:(hb + 1) * QB],
                            start=(kc == 0), stop=(kc == SH - 1),
                            skip_group_check=True)
                for pr in range(NPAIR):
                    for hb in range(2):
                        h = 2 * pr + hb
                        nc.tensor.matmul(
                            rs_ps[32 * h:32 * h + 1, :],
                            ones_t[:, 0:1],
                            pms[pr][:, hb * QB:(hb + 1) * QB],
                            start=(kc == 0), stop=(kc == SH - 1),
                            skip_group_check=True,
                            tile_position=(0, 32 * h))

            rb_pend = {}

            def normalize(qb):
                """r = 1/rowsum, broadcast via DRAM round trip. The xn
                multiplies are deferred (emit_xn) so the DVE queue is not
                head-of-line blocked on the DMA round trip."""
                r32 = rfp.tile([P, QB], F32, tag="r32", name="r32")
                r16 = rfp.tile([P, QB], BF16, tag="r16", name="r16")
                nc.vector.reciprocal_approx_fast(out=r32[:], in_=rs_ps[:])
                nc.vector.tensor_copy(r16[:], r32[:])
                rows = r16[:].rearrange("(g p) q -> g p q", p=32)[:, 0:1, :]
                nc.sync.dma_start(
                    rb_d[qb * NH:(qb + 1) * NH, :].rearrange(
                        "(g o) q -> g o q", o=1),
                    rows)
                rbs = []
                for pr in range(NPAIR):
                    rb = rbp.tile([P, QB], BF16, tag="rb", name="rb")
                    for hb in range(2):
                        row = qb * NH + 2 * pr + hb
                        nc.sync.dma_start(
                            rb[64 * hb:64 * hb + 64, :],
                            rb_d[row:row + 1, :].broadcast_to([64, QB]))
                    rbs.append(rb)
                rb_pend[qb] = rbs

            def emit_xn(qb):
                if qb not in rb_pend:
                    return
                qsl = slice(qb * QB, (qb + 1) * QB)
                rbs = rb_pend.pop(qb)
                for pr in range(NPAIR):
                    nc.vector.tensor_mul(
                        xn_t[pr][:, qsl], x_ps[pr][:], rbs[pr][:])

            def oproj_chunk(qb, hc, pool, evac=None):
                emit_xn(qb)
                qsl = slice(qb * QB, (qb + 1) * QB)
                if pool is yps:
                    y_ps = pool.tile([P, QB], F32, tag="y", name="y")
                else:
                    y_ps = pool.tile([P, 2 * QB], F32, tag="s",
                                     name="y")[:, 0:QB]
                for pr in range(NPAIR):
                    nc.tensor.matmul(
                        y_ps[:],
                        wo_t[pr][:, hc * P:(hc + 1) * P],
                        xn_t[pr][:, qsl],
                        start=(pr == 0), stop=(pr == NPAIR - 1))
                y_sb = ysb.tile([P, QB], BF16, tag="ysb", name="ysb")
                if evac is nc.scalar:
                    nc.scalar.copy(y_sb[:], y_ps[:])
                else:
                    nc.vector.tensor_copy(y_sb[:], y_ps[:])
                (nc.sync if hc % 2 == 0 else nc.gpsimd).dma_start(
                    y_d[hc * P:(hc + 1) * P, qsl], y_sb[:])

            # ---------------- emission schedule ----------------
            proj_chunk(kT_c[0], wk_t, kh_t, 0, nc.scalar)
            proj_chunk(qT_c[0], wq_t, qh_t, 0, nc.scalar)

            # (qb, kc) -> fused PE filler emitted just before that QK;
            # proj chunks split per-dc to halve the PE spike.
            filler = {
                (0, 1): lambda: late_loads(),
                (0, 2): lambda: proj_chunk(kT_c[1], wk_t, kh_t, 1, nc.vector,
                                           (0,)),
                (0, 3): lambda: proj_chunk(kT_c[1], wk_t, kh_t, 1, nc.vector,
                                           (1,)),
                (0, 6): lambda: proj_chunk(kT_c[2], wk_t, kh_t, 2, nc.vector,
                                           (0,)),
                (0, 7): lambda: proj_chunk(kT_c[2], wk_t, kh_t, 2, nc.vector,
                                           (1,)),
                (0, 10): lambda: proj_chunk(kT_c[3], wk_t, kh_t, 3, nc.vector,
                                            (0,)),
                (0, 11): lambda: proj_chunk(kT_c[3], wk_t, kh_t, 3, nc.vector,
                                            (1,)),
                (0, 13): lambda: proj_chunk(qT_c[1], wq_t, qh_t, 1, nc.vector,
                                            (0,)),
                (0, 14): lambda: proj_chunk(qT_c[1], wq_t, qh_t, 1, nc.vector,
                                            (1,)),
                (1, 2): lambda: proj_chunk(qT_c[2], wq_t, qh_t, 2, nc.vector,
                                           (0,)),
                (1, 3): lambda: proj_chunk(qT_c[2], wq_t, qh_t, 2, nc.vector,
                                           (1,)),
                (1, 8): lambda: proj_chunk(qT_c[3], wq_t, qh_t, 3, nc.vector,
                                           (0,)),
                (1, 9): lambda: proj_chunk(qT_c[3], wq_t, qh_t, 3, nc.vector,
                                           (1,)),
            }

            pending = deque()
            oproj_q = deque()

            def pop_pending():
                qb0, kc0, pms0 = pending.popleft()
                if kc0 == 0:
                    emit_xn(qb0 - 1)
                emit_av_rs(qb0, kc0, pms0)
                if kc0 == SH - 1:
                    normalize(qb0)
                    for hc in range(HT):
                        oproj_q.append((qb0, hc))

            t = 0
            for qb in range(QBn):
                for kc in range(SH):
                    if (qb, kc) in filler:
                        filler[(qb, kc)]()
                    pms = emit_qk_exp_mask(qb, kc)
                    if qb == 0:
                        vh_proj(kc)
                    pending.append((qb, kc, pms))
                    while pending and (len(pending) > LAG
                                       or pending[0][0] != qb):
                        pop_pending()
                    if t % OPROJ_EVERY == 1 and oproj_q:
                        oproj_chunk(*oproj_q.popleft(), yps)
                    t += 1
            while pending:
                pop_pending()
            j = 0
            while oproj_q:
                oproj_chunk(*oproj_q.popleft(), sps,
                            evac=(nc.scalar if j % 2 == 0 else nc.vector))
                j += 1

    nc.compile()
    return nc


def make_in_maps(q, k, v, mask, Wq, bq, Wk, bk, Wv, bv, Wo,
                 n_cores=8, NH=4, DK=64, aug=False):
    bf = ml_dtypes.bfloat16
    B, S, HID = q.shape
    D = NH * DK
    n_hg = n_cores // B

    def with_aug(xT, bias_row):
        pad = np.zeros((P, xT.shape[1]), xT.dtype)
        pad[0, :] = bias_row
        return np.concatenate([xT, pad], axis=0)

    per_batch = {}
    for b in range(B):
        qT = np.ascontiguousarray(q[b].T).astype(bf)
        kT = np.ascontiguousarray(k[b].T).astype(bf)
        vT = np.ascontiguousarray(v[b].T).astype(bf)
        if aug:
            one = np.ones((S,), np.float32).astype(bf)
            qT, kT, vT = with_aug(qT, one), with_aug(kT, one), with_aug(vT, one)
        per_batch[b] = (qT, kT, vT,
                        np.ascontiguousarray(mask[b, 0].T != 0).astype(bf))

    in_maps = []
    for core in range(n_cores):
        b, hg = divmod(core, n_hg)
        hsl = slice(hg * D, (hg + 1) * D)
        wq = Wq[:, hsl].astype(bf)
        wk = Wk[:, hsl].astype(bf)
        wv = Wv[:, hsl].astype(bf)
        if aug:
            wq = with_aug(wq, bq[hsl].astype(bf))
            wk = with_aug(wk, bk[hsl].astype(bf))
            wv = with_aug(wv, bv[hsl].astype(bf))
        qT, kT, vT, mT = per_batch[b]
        in_maps.append(dict(
            qT=qT, kT=kT, vT=vT, maskT=mT,
            wq=np.ascontiguousarray(wq), wk=np.ascontiguousarray(wk),
            wv=np.ascontiguousarray(wv),
            wo=np.ascontiguousarray(Wo[hsl, :]).astype(bf),
        ))
    return in_maps


def combine_outputs(results, B, S, HID, bo, n_cores=8):
    n_hg = n_cores // B
    out = np.zeros((B, S, HID), np.float32)
    for core in range(n_cores):
        b = core // n_hg
        out[b] += results[core]["y"].astype(np.float32).T
    return out + bo.astype(np.float32)


def run_mha(q, k, v, mask, Wq, bq, Wk, bk, Wv, bv, Wo, bo, trace=False):
    from concourse.bass_utils import run_bass_kernel_spmd

    B, S, HID = q.shape
    n_cores = 8
    aug = bool(np.any(bq) or np.any(bk) or np.any(bv))
    key = (S, HID, aug)
    if key not in _PROGRAM_CACHE:
        _PROGRAM_CACHE[key] = build_mha_program(S=S, HID=HID, aug=aug)
    nc = _PROGRAM_CACHE[key]
    in_maps = make_in_maps(q, k, v, mask, Wq, bq, Wk, bk, Wv, bv, Wo,
                           n_cores=n_cores, aug=aug)
    res = run_bass_kernel_spmd(nc, in_maps, list(range(n_cores)), trace=trace)
    out = combine_outputs(res.results, B, S, HID, bo, n_cores=n_cores)
    return out, res


def kernel(q, k, v, mask, Wq, bq, Wk, bk, Wv, bv, Wo, bo):
    q = np.asarray(q, np.float32)
    k = np.asarray(k, np.float32)
    v = np.asarray(v, np.float32)
    mask = np.asarray(mask)
    out, _ = run_mha(q, k, v, mask,
                     np.asarray(Wq, np.float32), np.asarray(bq, np.float32),
                     np.asarray(Wk, np.float32), np.asarray(bk, np.float32),
                     np.asarray(Wv, np.float32), np.asarray(bv, np.float32),
                     np.asarray(Wo, np.float32), np.asarray(bo, np.float32))
    return out


# revision 36
# speedup vs baseline: 1.1495x; 1.0115x over previous
"""TRN2 Bass kernel: 16-head MHA (B=2, S=2048, H=1024) sharded over 8 NeuronCores.

Sharding: data-parallel over batch (2) x tensor-parallel over head groups
(4 groups of 4 heads). Each core computes its 4 heads' attention for its batch
and a partial output projection; the host sums the 4 partials per batch,
transposes, and adds the output bias.

v3: fully fused single-phase pipeline.
  - QK^T head pairs issued as concurrent row-tiles ((0,0)/(64,0)); AV pairs as
    col-tiles ((0,0)/(0,64)); softmax denominators from a 4-way col-tiled pass
    of M=1 ones-matmuls accumulating in a dedicated PSUM bank.
  - exp() is one [128, 1024] ACTIVATE per (qb, kc, pair); 1/rowsum via
    reciprocal_approx_fast; partition-broadcast via a bf16 DRAM round trip;
    normalization multiplies PSUM x directly (no intermediate xu).
  - Projections are fused into the attention stream: inputs arrive as
    column-chunk DMAs on three queues; Q/K projection chunks and per-kc V
    projections run as PE filler inside q-block 0/1/2, sharing one PSUM bank
    ring with the output projection.
  - Output projection chunks trail one q-block behind; y is written bf16
    (host sums the 4 partials per batch in fp32).
"""

import sys

sys.path.insert(0, "/opt/trn_rl_repo")

from collections import deque
from contextlib import ExitStack

import numpy as np
import ml_dtypes

import concourse.tile as tile
from concourse import bacc, mybir

BF16 = mybir.dt.bfloat16
F32 = mybir.dt.float32
P = 128

LAG = 4            # kc-instances by which AV/rowsum matmuls trail QK/exp/mask
USE_GPSIMD_MASK = True   # offload 1/4 of mask multiplies to GPSIMD
OPROJ_EVERY = 2    # pop one oproj chunk every N kc-instances

_PROGRAM_CACHE = {}


def build_mha_program(S=2048, HID=1024, NH=4, DK=64, QB=512, aug=False):
    """Build + compile the per-core SPMD Bass program."""
    D = NH * DK
    assert NH == 4 and DK == 64
    SH = S // P                 # 16 key blocks
    HT = HID // P               # 8 hidden blocks
    HTa = HT + (1 if aug else 0)
    QBn = S // QB               # 4 q-blocks
    NPAIR = NH // 2             # 2 head pairs
    CH = S // QB                # 4 input column chunks (same size as QB)

    nc = bacc.Bacc("TRN2", target_bir_lowering=False, debug=False)

    qT_d = nc.dram_tensor("qT", [HTa * P, S], BF16, kind="ExternalInput").ap()
    kT_d = nc.dram_tensor("kT", [HTa * P, S], BF16, kind="ExternalInput").ap()
    vT_d = nc.dram_tensor("vT", [HTa * P, S], BF16, kind="ExternalInput").ap()
    maskT_d = nc.dram_tensor("maskT", [S, S], BF16, kind="ExternalInput").ap()
    wq_d = nc.dram_tensor("wq", [HTa * P, D], BF16, kind="ExternalInput").ap()
    wk_d = nc.dram_tensor("wk", [HTa * P, D], BF16, kind="ExternalInput").ap()
    wv_d = nc.dram_tensor("wv", [HTa * P, D], BF16, kind="ExternalInput").ap()
    wo_d = nc.dram_tensor("wo", [D, HID], BF16, kind="ExternalInput").ap()
    y_d = nc.dram_tensor("y", [HID, S], BF16, kind="ExternalOutput").ap()
    # DRAM bounce buffer for partition-broadcasting the softmax reciprocals
    rb_d = nc.dram_tensor("r_bounce", [NH * QBn, QB], BF16).ap()

    Exp = mybir.ActivationFunctionType.Exp

    with tile.TileContext(nc) as tc:
        with ExitStack() as ctx:
            persist = ctx.enter_context(tc.tile_pool(name="persist", bufs=1))
            wpool = ctx.enter_context(tc.tile_pool(name="wpool", bufs=1))
            inq = ctx.enter_context(tc.tile_pool(name="inq", bufs=2))
            mp = ctx.enter_context(tc.tile_pool(name="mask", bufs=1))
            pp = ctx.enter_context(tc.tile_pool(name="pexp", bufs=2))
            pmp = ctx.enter_context(
                tc.tile_pool(name="pmask", bufs=2 * (LAG + 2)))
            rfp = ctx.enter_context(tc.tile_pool(name="rfp", bufs=2))
            rbp = ctx.enter_context(tc.tile_pool(name="rbp", bufs=2))
            ysb = ctx.enter_context(tc.tile_pool(name="ysb", bufs=2))
            sps = ctx.enter_context(
                tc.tile_pool(name="sps", bufs=2, space="PSUM"))
            xps = ctx.enter_context(
                tc.tile_pool(name="xps", bufs=1, space="PSUM"))
            rsps = ctx.enter_context(
                tc.tile_pool(name="rsps", bufs=1, space="PSUM"))
            yps = ctx.enter_context(
                tc.tile_pool(name="yps", bufs=1, space="PSUM"))

            qh_t = [persist.tile([P, S], BF16, tag=f"qh{d}", name=f"qh{d}")
                    for d in range(NPAIR)]
            kh_t = [persist.tile([P, S], BF16, tag=f"kh{d}", name=f"kh{d}")
                    for d in range(NPAIR)]
            vh_t = [persist.tile([P, D], BF16, tag=f"vh{s}", name=f"vh{s}")
                    for s in range(SH)]
            xn_t = [persist.tile([P, S], BF16, tag=f"xn{p}", name=f"xn{p}")
                    for p in range(NPAIR)]
            wo_t = [persist.tile([P, HID], BF16, tag=f"wo{p}", name=f"wo{p}")
                    for p in range(NPAIR)]
            ones_t = persist.tile([P, 4], BF16, tag="ones", name="ones")
            nc.vector.memset(ones_t[:], 1.0)

            wq_t = wpool.tile([P, HTa * D], BF16, tag="wq", name="wq")
            wk_t = wpool.tile([P, HTa * D], BF16, tag="wk", name="wk")
            wv_t = wpool.tile([P, HTa * D], BF16, tag="wv", name="wv")

            mask_t = [mp.tile([P, S], BF16, tag=f"m{i}", name=f"m{i}")
                      for i in range(SH)]

            x_ps = [xps.tile([P, QB], F32, tag=f"x{p}", name=f"x{p}")
                    for p in range(NPAIR)]
            rs_ps = rsps.tile([P, QB], F32, tag="rs", name="rs")

            def wload(eng, dst_t, src_d):
                dst = dst_t[:].rearrange("p (i s) -> p i s", s=D)
                src = src_d[:, :].rearrange("(i p) s -> p i s", p=P)
                eng.dma_start(dst, src)

            def chunk_load(eng, tag, src_d, c):
                """Column chunk c (QB cols) of all HTa row-blocks."""
                t = inq.tile([P, HTa * QB], BF16, tag=tag, name=f"{tag}{c}")
                dst = t[:].rearrange("p (i s) -> p i s", s=QB)
                src = src_d[:, c * QB:(c + 1) * QB].rearrange(
                    "(i p) s -> p i s", p=P)
                eng.dma_start(dst, src)
                return t

            # -------- DMA schedule (3 queues: sync / scalar / gpsimd) -------
            # Head of pipeline: kT0 alone on scalar (gates kh-c0 proj),
            # wq+qT0 on sync (gates qh-c0 proj), wk/wv early on gpsimd.
            qT_c, kT_c, vT_c = {}, {}, {}
            kT_c[0] = chunk_load(nc.scalar, "kT", kT_d, 0)
            wload(nc.sync, wq_t, wq_d)
            qT_c[0] = chunk_load(nc.sync, "qT", qT_d, 0)
            wload(nc.gpsimd, wk_t, wk_d)
            wload(nc.gpsimd, wv_t, wv_d)
            kT_c[1] = chunk_load(nc.scalar, "kT", kT_d, 1)
            nc.sync.dma_start(mask_t[0][:], maskT_d[0:P, :])
            vT_c[0] = chunk_load(nc.gpsimd, "vT", vT_d, 0)
            nc.sync.dma_start(mask_t[1][:], maskT_d[P:2 * P, :])
            qT_c[1] = chunk_load(nc.scalar, "qT", qT_d, 1)
            vT_c[1] = chunk_load(nc.gpsimd, "vT", vT_d, 1)
            nc.gpsimd.dma_start(wo_t[0][:], wo_d[0:P, :])
            nc.gpsimd.dma_start(wo_t[1][:], wo_d[P:2 * P, :])
            for i in range(2, 8):
                (nc.gpsimd if i % 2 == 0 else nc.sync).dma_start(
                    mask_t[i][:], maskT_d[i * P:(i + 1) * P, :])

            def mload(eng, i):
                eng.dma_start(mask_t[i][:], maskT_d[i * P:(i + 1) * P, :])

            def late_loads():
                """Issued mid-stream on sync/gpsimd (never scalar: its queue
                carries the exp stream). Later tranches are staggered so the
                latency-critical rb/y DMAs interleave between them."""
                kT_c[2] = chunk_load(nc.gpsimd, "kT", kT_d, 2)
                vT_c[2] = chunk_load(nc.sync, "vT", vT_d, 2)
                kT_c[3] = chunk_load(nc.gpsimd, "kT", kT_d, 3)
                qT_c[2] = chunk_load(nc.scalar, "qT", qT_d, 2)
                vT_c[3] = chunk_load(nc.gpsimd, "vT", vT_d, 3)
                qT_c[3] = chunk_load(nc.scalar, "qT", qT_d, 3)
                for i in range(8, SH):
                    mload(nc.sync if i % 2 == 0 else nc.gpsimd, i)

            # ---------------- fused projection helpers ----------------
            def proj_chunk(src_c, w_t, dst, qc, eng, dcs=(0, 1)):
                """dst[:, qc chunk] = (w.T @ src) for the given row halves."""
                for dc in dcs:
                    ps = yps.tile([P, QB], F32, tag="y", name=f"pj{qc}{dc}")
                    for i in range(HTa):
                        nc.tensor.matmul(
                            ps[:],
                            w_t[:, i * D + dc * P:i * D + (dc + 1) * P],
                            src_c[:, i * QB:(i + 1) * QB],
                            start=(i == 0), stop=(i == HTa - 1))
                    dst_ap = dst[dc][:, qc * QB:(qc + 1) * QB]
                    if eng is nc.scalar:
                        eng.copy(dst_ap, ps[:])
                    else:
                        eng.tensor_copy(dst_ap, ps[:])

            def vh_proj(sc):
                """vh[sc] = vT[:, sc block].T @ wv  (one [P, D] tile)."""
                c = sc // (QB // P)
                off = (sc % (QB // P)) * P
                ps = yps.tile([P, QB], F32, tag="y", name=f"vj{sc}")
                for i in range(HTa):
                    nc.tensor.matmul(
                        ps[:, 0:D],
                        vT_c[c][:, i * QB + off:i * QB + off + P],
                        wv_t[:, i * D:(i + 1) * D],
                        start=(i == 0), stop=(i == HTa - 1))
                nc.vector.tensor_copy(vh_t[sc][:], ps[:, 0:D])

            # ---------------- attention pipeline ----------------
            def emit_qk_exp_mask(qb, kc):
                qsl = slice(qb * QB, (qb + 1) * QB)
                pms = []
                for pr in range(NPAIR):
                    s_ps = sps.tile([P, 2 * QB], F32, tag="s", name="s")
                    for hb in range(2):
                        rsl = slice(64 * hb, 64 * hb + 64)
                        nc.tensor.matmul(
                            s_ps[:, hb * QB:(hb + 1) * QB],
                            kh_t[pr][rsl, kc * P:(kc + 1) * P],
                            qh_t[pr][rsl, qsl],
                            start=True, stop=True)
                    p_t = pp.tile([P, 2 * QB], BF16, tag="p", name="p")
                    nc.scalar.activation(p_t[:], s_ps[:], Exp, scale=0.125)
                    pm = pmp.tile([P, 2 * QB], BF16, tag="pm", name="pm")
                    eng = nc.vector
                    if USE_GPSIMD_MASK and (pr == 1) and (kc % 2 == 1):
                        eng = nc.gpsimd
                    for hb in range(2):
                        eng.tensor_mul(
                            pm[:, hb * QB:(hb + 1) * QB],
                            p_t[:, hb * QB:(hb + 1) * QB],
                            mask_t[kc][:, qsl])
                    pms.append(pm)
                return pms

            def emit_av_rs(qb, kc, pms):
                for pr in range(NPAIR):
                    for hb in range(2):
                        h = 2 * pr + hb
                        nc.tensor.matmul(
                            x_ps[pr][64 * hb:64 * hb + 64, :],
                            vh_t[kc][:, h * DK:(h + 1) * DK],
                            pms[pr][:, hb * QB:(hb + 1) * QB],
                            start=(kc == 0), stop=(kc == SH - 1),
                            skip_group_check=True)
                for pr in range(NPAIR):
                    for hb in range(2):
                        h = 2 * pr + hb
                        nc.tensor.matmul(
                            rs_ps[32 * h:32 * h + 1, :],
                            ones_t[:, 0:1],
                            pms[pr][:, hb * QB:(hb + 1) * QB],
                            start=(kc == 0), stop=(kc == SH - 1),
                            skip_group_check=True,
                            tile_position=(0, 32 * h))

            rb_pend = {}

            def normalize(qb):
                """r = 1/rowsum, broadcast via DRAM round trip. The xn
                multiplies are deferred (emit_xn) so the DVE queue is not
                head-of-line blocked on the DMA round trip."""
                r32 = rfp.tile([P, QB], F32, tag="r32", name="r32")
                r16 = rfp.tile([P, QB], BF16, tag="r16", name="r16")
                nc.vector.reciprocal_approx_fast(out=r32[:], in_=rs_ps[:])
                nc.vector.tensor_copy(r16[:], r32[:])
                rows = r16[:].rearrange("(g p) q -> g p q", p=32)[:, 0:1, :]
                nc.sync.dma_start(
                    rb_d[qb * NH:(qb + 1) * NH, :].rearrange(
                        "(g o) q -> g o q", o=1),
                    rows)
                rbs = []
                for pr in range(NPAIR):
                    rb = rbp.tile([P, QB], BF16, tag="rb", name="rb")
                    for hb in range(2):
                        row = qb * NH + 2 * pr + hb
                        nc.sync.dma_start(
                            rb[64 * hb:64 * hb + 64, :],
                            rb_d[row:row + 1, :].broadcast_to([64, QB]))
                    rbs.append(rb)
                rb_pend[qb] = rbs

            def emit_xn(qb):
                if qb not in rb_pend:
                    return
                qsl = slice(qb * QB, (qb + 1) * QB)
                rbs = rb_pend.pop(qb)
                for pr in range(NPAIR):
                    nc.vector.tensor_mul(
                        xn_t[pr][:, qsl], x_ps[pr][:], rbs[pr][:])

            def oproj_chunk(qb, hc, pool, evac=None):
                emit_xn(qb)
                qsl = slice(qb * QB, (qb + 1) * QB)
                if pool is yps:
                    y_ps = pool.tile([P, QB], F32, tag="y", name="y")
                else:
                    y_ps = pool.tile([P, 2 * QB], F32, tag="s",
                                     name="y")[:, 0:QB]
                for pr in range(NPAIR):
                    nc.tensor.matmul(
                        y_ps[:],
                        wo_t[pr][:, hc * P:(hc + 1) * P],
                        xn_t[pr][:, qsl],
                        start=(pr == 0), stop=(pr == NPAIR - 1))
                y_sb = ysb.tile([P, QB], BF16, tag="ysb", name="ysb")
                if evac is nc.scalar:
                    nc.scalar.copy(y_sb[:], y_ps[:])
                else:
                    nc.vector.tensor_copy(y_sb[:], y_ps[:])
                (nc.sync if hc % 2 == 0 else nc.gpsimd).dma_start(
                    y_d[hc * P:(hc + 1) * P, qsl], y_sb[:])

            # ---------------- emission schedule ----------------
            proj_chunk(kT_c[0], wk_t, kh_t, 0, nc.scalar)
            proj_chunk(qT_c[0], wq_t, qh_t, 0, nc.scalar)

            # (qb, kc) -> fused PE filler emitted just before that QK;
            # proj chunks split per-dc to halve the PE spike.
            filler = {
                (0, 1): lambda: late_loads(),
                (0, 2): lambda: proj_chunk(kT_c[1], wk_t, kh_t, 1, nc.vector,
                                           (0,)),
                (0, 3): lambda: proj_chunk(kT_c[1], wk_t, kh_t, 1, nc.vector,
                                           (1,)),
                (0, 6): lambda: proj_chunk(kT_c[2], wk_t, kh_t, 2, nc.vector,
                                           (0,)),
                (0, 7): lambda: proj_chunk(kT_c[2], wk_t, kh_t, 2, nc.vector,
                                           (1,)),
                (0, 10): lambda: proj_chunk(kT_c[3], wk_t, kh_t, 3, nc.vector,
                                            (0,)),
                (0, 11): lambda: proj_chunk(kT_c[3], wk_t, kh_t, 3, nc.vector,
                                            (1,)),
                (0, 13): lambda: proj_chunk(qT_c[1], wq_t, qh_t, 1, nc.vector,
                                            (0,)),
                (0, 14): lambda: proj_chunk(qT_c[1], wq_t, qh_t, 1, nc.vector,
                                            (1,)),
                (1, 6): lambda: proj_chunk(qT_c[2], wq_t, qh_t, 2, nc.vector,
                                           (0,)),
                (1, 7): lambda: proj_chunk(qT_c[2], wq_t, qh_t, 2, nc.vector,
                                           (1,)),
                (2, 2): lambda: proj_chunk(qT_c[3], wq_t, qh_t, 3, nc.vector,
                                           (0,)),
                (2, 3): lambda: proj_chunk(qT_c[3], wq_t, qh_t, 3, nc.vector,
                                           (1,)),
            }

            pending = deque()
            oproj_q = deque()

            def pop_pending():
                qb0, kc0, pms0 = pending.popleft()
                if kc0 == 0:
                    emit_xn(qb0 - 1)
                emit_av_rs(qb0, kc0, pms0)
                if kc0 == SH - 1:
                    normalize(qb0)
                    for hc in range(HT):
                        oproj_q.append((qb0, hc))

            t = 0
            for qb in range(QBn):
                for kc in range(SH):
                    if (qb, kc) in filler:
                        filler[(qb, kc)]()
                    pms = emit_qk_exp_mask(qb, kc)
                    if qb == 0:
                        vh_proj(kc)
                    pending.append((qb, kc, pms))
                    while pending and (len(pending) > LAG
                                       or pending[0][0] != qb):
                        pop_pending()
                    if t % OPROJ_EVERY == 1 and oproj_q:
                        oproj_chunk(*oproj_q.popleft(), yps)
                    t += 1
            while pending:
                pop_pending()
            emit_xn(QBn - 1)
            j = 0
            while oproj_q:
                oproj_chunk(*oproj_q.popleft(),
                            sps if j % 2 == 0 else yps,
                            evac=(nc.scalar if j % 2 == 0 else nc.vector))
                j += 1

    nc.compile()
    return nc


def make_in_maps(q, k, v, mask, Wq, bq, Wk, bk, Wv, bv, Wo,
                 n_cores=8, NH=4, DK=64, aug=False):
    bf = ml_dtypes.bfloat16
    B, S, HID = q.shape
    D = NH * DK
    n_hg = n_cores // B

    def with_aug(xT, bias_row):
        pad = np.zeros((P, xT.shape[1]), xT.dtype)
        pad[0, :] = bias_row
        return np.concatenate([xT, pad], axis=0)

    per_batch = {}
    for b in range(B):
        qT = np.ascontiguousarray(q[b].T).astype(bf)
        kT = np.ascontiguousarray(k[b].T).astype(bf)
        vT = np.ascontiguousarray(v[b].T).astype(bf)
        if aug:
            one = np.ones((S,), np.float32).astype(bf)
            qT, kT, vT = with_aug(qT, one), with_aug(kT, one), with_aug(vT, one)
        per_batch[b] = (qT, kT, vT,
                        np.ascontiguousarray(mask[b, 0].T != 0).astype(bf))

    in_maps = []
    for core in range(n_cores):
        b, hg = divmod(core, n_hg)
        hsl = slice(hg * D, (hg + 1) * D)
        wq = Wq[:, hsl].astype(bf)
        wk = Wk[:, hsl].astype(bf)
        wv = Wv[:, hsl].astype(bf)
        if aug:
            wq = with_aug(wq, bq[hsl].astype(bf))
            wk = with_aug(wk, bk[hsl].astype(bf))
            wv = with_aug(wv, bv[hsl].astype(bf))
        qT, kT, vT, mT = per_batch[b]
        in_maps.append(dict(
            qT=qT, kT=kT, vT=vT, maskT=mT,
            wq=np.ascontiguousarray(wq), wk=np.ascontiguousarray(wk),
            wv=np.ascontiguousarray(wv),
            wo=np.ascontiguousarray(Wo[hsl, :]).astype(bf),
        ))
    return in_maps


def combine_outputs(results, B, S, HID, bo, n_cores=8):
    n_hg = n_cores // B
    out = np.zeros((B, S, HID), np.float32)
    for core in range(n_cores):
        b = core // n_hg
        out[b] += results[core]["y"].astype(np.float32).T
    return out + bo.astype(np.float32)


def run_mha(q, k, v, mask, Wq, bq, Wk, bk, Wv, bv, Wo, bo, trace=False):
    from concourse.bass_utils import run_bass_kernel_spmd

    B, S, HID = q.shape
    n_cores = 8
    aug = bool(np.any(bq) or np.any(bk) or np.any(bv))
    key = (S, HID, aug)
    if key not in _PROGRAM_CACHE:
        _PROGRAM_CACHE[key] = build_mha_program(S=S, HID=HID, aug=aug)
    nc = _PROGRAM_CACHE[key]
    in_maps = make_in_maps(q, k, v, mask, Wq, bq, Wk, bk, Wv, bv, Wo,
                           n_cores=n_cores, aug=aug)
    res = run_bass_kernel_spmd(nc, in_maps, list(range(n_cores)), trace=trace)
    out = combine_outputs(res.results, B, S, HID, bo, n_cores=n_cores)
    return out, res


def kernel(q, k, v, mask, Wq, bq, Wk, bk, Wv, bv, Wo, bo):
    q = np.asarray(q, np.float32)
    k = np.asarray(k, np.float32)
    v = np.asarray(v, np.float32)
    mask = np.asarray(mask)
    out, _ = run_mha(q, k, v, mask,
                     np.asarray(Wq, np.float32), np.asarray(bq, np.float32),
                     np.asarray(Wk, np.float32), np.asarray(bk, np.float32),
                     np.asarray(Wv, np.float32), np.asarray(bv, np.float32),
                     np.asarray(Wo, np.float32), np.asarray(bo, np.float32))
    return out
